# revision 1
# baseline (speedup 1.0000x reference)
"""Trainium2 Bass kernel for a dense transformer block (LN -> 16-head causal
attention -> residual -> LN -> FFN -> residual) on x:(2, 2048, 1024) fp32.

Sharding: 8 cores, zero collectives. Core c handles batch b=c//4, query chunk
a=c%4 (512 contiguous tokens). Every core recomputes full-sequence K/V for its
batch from a replicated (transposed) copy of x[b]; attention for the 512
queries runs against all 2048 keys with an additive causal mask supplied as
per-core input data, so the compiled program is identical across cores (SPMD).

Everything on-chip runs feature-on-partition ("T-layout"): LayerNorm statistics
are partition reductions done with ones-vector matmuls, the softmax denominator
comes from a ones column appended to V, and per-token stats are broadcast back
across partitions with gpsimd.partition_broadcast. Matmuls run in bf16 with
fp32 PSUM accumulation; both residual adds are carried in fp32.
"""

import numpy as np
import ml_dtypes

import concourse.bass as bass
import concourse.tile as tile
from concourse import bacc, mybir
from concourse import bass_utils
from concourse.bass import ts

P = 128
B, T, C = 2, 2048, 1024
H, D = 16, 64
FF = 4 * C
CC = C // P          # 8 feature chunks
TQ = 512             # queries per core
NSCH = T // P        # 16 key chunks
EPS = 1e-5
NEG = -30000.0
bf16 = ml_dtypes.bfloat16

f32 = mybir.dt.float32
bf = mybir.dt.bfloat16
AF = mybir.ActivationFunctionType
ALU = mybir.AluOpType


def _ln_T(nc, big1, chunked, spsum, x_tile, Tn, out_tile, g_sb, be_sb, eps11,
          ones1, x_is_f32):
    """LayerNorm over the feature dim with activations feature-on-partition.
    x_tile/out_tile: (P, CC, Tn). Stats via ones-matmul partition reduction,
    processed 512 tokens at a time. big1: bufs=1 pool; chunked: bufs>=2."""
    for tch in range(Tn // 512):
        xs = x_tile[:, :, ts(tch, 512)]
        ps = spsum.tile([1, 512], f32, tag="stat")
        if x_is_f32:
            for cc in range(CC):
                xbf = chunked.tile([P, 512], bf, tag="ln_xbf")
                nc.vector.tensor_copy(xbf, xs[:, cc, :])
                nc.tensor.matmul(ps, lhsT=ones1, rhs=xbf,
                                 start=(cc == 0), stop=(cc == CC - 1))
        else:
            for cc in range(CC):
                nc.tensor.matmul(ps, lhsT=ones1, rhs=xs[:, cc, :],
                                 start=(cc == 0), stop=(cc == CC - 1))
        pq = spsum.tile([1, 512], f32, tag="stat")
        for cc in range(CC):
            sq = chunked.tile([P, 512], bf, tag="ln_sq")
            nc.vector.tensor_mul(sq, xs[:, cc, :], xs[:, cc, :])
            nc.tensor.matmul(pq, lhsT=ones1, rhs=sq,
                             start=(cc == 0), stop=(cc == CC - 1))
        m = big1.tile([1, 512], f32, tag="ln_m")
        nc.vector.tensor_scalar_mul(m, ps, 1.0 / C)
        q = big1.tile([1, 512], f32, tag="ln_q")
        nc.vector.tensor_scalar_mul(q, pq, 1.0 / C)
        msq = big1.tile([1, 512], f32, tag="ln_msq")
        nc.vector.tensor_mul(msq, m, m)
        nc.vector.tensor_tensor(q, q, msq, ALU.subtract)  # q := var
        sd = big1.tile([1, 512], f32, tag="ln_sd")
        nc.scalar.activation(sd, q, AF.Sqrt, bias=eps11)
        a_t = big1.tile([1, 512], f32, tag="ln_at")
        nc.vector.reciprocal(a_t, sd)
        b_t = big1.tile([1, 512], f32, tag="ln_bt")
        nc.vector.tensor_mul(b_t, m, a_t)

        a_bc = chunked.tile([P, 512], f32, tag="ln_abc")
        nc.gpsimd.partition_broadcast(a_bc, a_t)
        b_bc = chunked.tile([P, 512], f32, tag="ln_bbc")
        nc.gpsimd.partition_broadcast(b_bc, b_t)
        for cc in range(CC):
            t1 = chunked.tile([P, 512], bf, tag="ln_t1")
            nc.vector.tensor_mul(t1, xs[:, cc, :], a_bc)
            nc.vector.tensor_tensor(t1, t1, b_bc, ALU.subtract)
            nc.vector.tensor_scalar(out_tile[:, cc, ts(tch, 512)], t1,
                                    scalar1=g_sb[:, cc:cc + 1],
                                    scalar2=be_sb[:, cc:cc + 1],
                                    op0=ALU.mult, op1=ALU.add)


FKV = 8 * 512 + 4 * H * 65          # AllGather payload per core (bf16 elems)


def _body(nc, tc, aps, use_ag, bounces):
    (xkvT, xqT, maskT, wq, wk, wv, wo, w1, w2,
     bo_t, b1_t, b2_t, g1_t, be1_t, g2_t, be2_t, outT) = aps

    import contextlib
    ctx = contextlib.ExitStack()
    with ctx:
        # pools that live for the whole kernel (small stuff + psum)
        consts = ctx.enter_context(tc.tile_pool(name="consts", bufs=1))
        small = ctx.enter_context(tc.tile_pool(name="small", bufs=2))
        ppool = ctx.enter_context(tc.tile_pool(name="ppool", bufs=5, space="PSUM"))
        opsum = ctx.enter_context(tc.tile_pool(name="opsum", bufs=1, space="PSUM"))
        spsum = ctx.enter_context(tc.tile_pool(name="spsum", bufs=2, space="PSUM"))

        ones1 = consts.tile([P, 1], bf)
        nc.vector.memset(ones1, 1.0)
        eps11 = consts.tile([1, 1], f32)
        nc.vector.memset(eps11, EPS)

        def load(pool, ap_dram, shape, dtype=f32, tag=None):
            t = pool.tile(list(shape), dtype, tag=tag or ap_dram.name)
            nc.sync.dma_start(t, ap_dram)
            return t

        bo_s = load(consts, bo_t, (P, CC))
        b1_s = load(consts, b1_t, (P, 32))
        b2_s = load(consts, b2_t, (P, CC))
        g1_s = load(consts, g1_t, (P, CC))
        be1_s = load(consts, be1_t, (P, CC))
        g2_s = load(consts, g2_t, (P, CC))
        be2_s = load(consts, be2_t, (P, CC))

        # ---- OT survives until the output projection (opened first: LIFO)
        opool = ctx.enter_context(tc.tile_pool(name="opool", bufs=1))

        # ---- KT/Vr/QT live from the projections to the end of attention
        kvq_ctx = contextlib.ExitStack()
        kvq = kvq_ctx.enter_context(tc.tile_pool(name="kvq", bufs=1))
        KT = kvq.tile([P, 8, T], bf)
        Vr = kvq.tile([P, NSCH, H, 65], bf)
        QT = kvq.tile([P, 8, TQ], bf)

        # ---- hkv/hq live until the end of the QKV projections
        with tc.tile_pool(name="hpool", bufs=1) as hpool:
            hq = hpool.tile([P, CC, TQ], bf)

            if not use_ag:
                hkv = hpool.tile([P, CC, T], bf)
                # LN1 over the full batch sequence (for K/V)
                with tc.tile_pool(name="p_ln1", bufs=1) as p1, \
                     tc.tile_pool(name="p_ln1b", bufs=1) as p1b, \
                     tc.tile_pool(name="p_ln1t", bufs=3) as p1t:
                    xkv_sb = load(p1, xkvT, (P, CC, T), bf)
                    _ln_T(nc, p1b, p1t, spsum, xkv_sb, T, hkv,
                          g1_s, be1_s, eps11, ones1, x_is_f32=False)

            # LN1 over the query slice
            with tc.tile_pool(name="p_ln1q", bufs=1) as p2, \
                 tc.tile_pool(name="p_ln1qb", bufs=1) as p2b, \
                 tc.tile_pool(name="p_ln1qt", bufs=2) as p2t:
                xq_sb = load(p2, xqT, (P, CC, TQ), f32, tag="xq_ln")
                _ln_T(nc, p2b, p2t, spsum, xq_sb, TQ, hq,
                      g1_s, be1_s, eps11, ones1, x_is_f32=True)

            with tc.tile_pool(name="p_w", bufs=1) as pw:
                wq_s = load(pw, wq, (P, CC, 8, P), bf)
                wk_s = load(pw, wk, (P, CC, 8, P), bf)
                wv_s = load(pw, wv, (P, CC, C), bf)

                if use_ag:
                    kv_in, kv_out = bounces
                    # own-chunk K^T (s = this core's 512 tokens)
                    KTo = pw.tile([P, CC, TQ], bf, tag="KTo")
                    for pair in range(8):
                        psum = ppool.tile([P, 512], f32, tag="mm")
                        for cc in range(CC):
                            nc.tensor.matmul(psum, lhsT=wk_s[:, cc, pair, :],
                                             rhs=hq[:, cc, :],
                                             start=(cc == 0), stop=(cc == CC - 1))
                        nc.vector.tensor_copy(KTo[:, pair, :], psum)
                    # own-chunk V rows (4 s-tiles) with ones column
                    Vro = pw.tile([P, 4, H, 65], bf, tag="Vro")
                    nc.vector.memset(Vro[:, :, :, 64:65], 1.0)
                    for st in range(4):
                        for half in range(2):
                            psum = ppool.tile([P, 512], f32, tag="mm")
                            for cc in range(CC):
                                nc.tensor.matmul(psum, lhsT=hq[:, cc, ts(st, P)],
                                                 rhs=wv_s[:, cc, ts(half, 512)],
                                                 start=(cc == 0), stop=(cc == CC - 1))
                            nc.vector.tensor_copy(
                                Vro[:, st, half * 8:(half + 1) * 8, 0:64],
                                psum.rearrange("p (h d) -> p h d", d=64))
                    # bounce out, AllGather within the 4-core batch group,
                    # then scatter the gathered chunks into KT / Vr
                    nc.sync.dma_start(
                        kv_in.ap()[:, 0:4096].rearrange("p (a b) -> p a b", a=CC),
                        KTo)
                    nc.sync.dma_start(
                        kv_in.ap()[:, 4096:FKV].rearrange(
                            "p (a h e) -> p a h e", a=4, h=H), Vro)
                    nc.gpsimd.collective_compute(
                        "AllGather",
                        mybir.AluOpType.bypass,
                        replica_groups=[[0, 1, 2, 3], [4, 5, 6, 7]],
                        ins=[kv_in.ap().opt()],
                        outs=[kv_out.ap().opt()],
                    )
                    for r in range(4):
                        blk = kv_out.ap()[r * P:(r + 1) * P, :]
                        nc.sync.dma_start(
                            KT[:, :, ts(r, 512)],
                            blk[:, 0:4096].rearrange("p (a b) -> p a b", a=CC))
                        nc.sync.dma_start(
                            Vr[:, 4 * r:4 * (r + 1), :, :],
                            blk[:, 4096:FKV].rearrange(
                                "p (a h e) -> p a h e", a=4, h=H))
                else:
                    for pair in range(8):
                        for sch in range(4):
                            psum = ppool.tile([P, 512], f32, tag="mm")
                            for cc in range(CC):
                                nc.tensor.matmul(psum, lhsT=wk_s[:, cc, pair, :],
                                                 rhs=hkv[:, cc, ts(sch, 512)],
                                                 start=(cc == 0), stop=(cc == CC - 1))
                            nc.vector.tensor_copy(KT[:, pair, ts(sch, 512)], psum)

                    nc.vector.memset(Vr[:, :, :, 64:65], 1.0)
                    for st in range(NSCH):
                        for half in range(2):
                            psum = ppool.tile([P, 512], f32, tag="mm")
                            for cc in range(CC):
                                nc.tensor.matmul(psum, lhsT=hkv[:, cc, ts(st, P)],
                                                 rhs=wv_s[:, cc, ts(half, 512)],
                                                 start=(cc == 0), stop=(cc == CC - 1))
                            nc.vector.tensor_copy(
                                Vr[:, st, half * 8:(half + 1) * 8, 0:64],
                                psum.rearrange("p (h d) -> p h d", d=64))

                for pair in range(8):
                    psum = ppool.tile([P, 512], f32, tag="mm")
                    for cc in range(CC):
                        nc.tensor.matmul(psum, lhsT=wq_s[:, cc, pair, :],
                                         rhs=hq[:, cc, :],
                                         start=(cc == 0), stop=(cc == CC - 1))
                    nc.vector.tensor_scalar_mul(QT[:, pair, :], psum,
                                                float(C) ** -0.5)

        # ---- attention
        OT = opool.tile([P, 8, TQ], bf)
        with tc.tile_pool(name="p_att", bufs=1) as pa, \
             tc.tile_pool(name="p_attt", bufs=4) as pat:
            mask_sb = load(pa, maskT, (P, NSCH, TQ), bf)
            for h in range(H):
                pair, half = h // 2, h % 2
                hp = slice(64 * half, 64 * half + 64)
                ops = opsum.tile([P, 512], f32, tag="av")
                for sch in range(NSCH):
                    sps = ppool.tile([P, 512], f32, tag="mm")
                    nc.tensor.matmul(sps, lhsT=KT[hp, pair, ts(sch, P)],
                                     rhs=QT[hp, pair, :], start=True, stop=True)
                    sm = pat.tile([P, TQ], bf, tag="sm")
                    nc.vector.tensor_tensor(sm, sps, mask_sb[:, sch, :], ALU.add)
                    e = pat.tile([P, TQ], bf, tag="e")
                    nc.scalar.activation(e, sm, AF.Exp)
                    nc.tensor.matmul(ops[0:65, :], lhsT=Vr[:, sch, h, :], rhs=e,
                                     start=(sch == 0), stop=(sch == NSCH - 1))
                zr = small.tile([1, TQ], f32, tag="zr")
                nc.vector.reciprocal(zr, ops[64:65, :])
                zb = pat.tile([64, TQ], f32, tag="zb")
                nc.gpsimd.partition_broadcast(zb, zr)
                nc.vector.tensor_mul(OT[hp, pair, :], ops[0:64, :], zb)
        kvq_ctx.close()

        # ---- output projection + bias + residual; LN2; FFN
        with tc.tile_pool(name="p_ffn", bufs=1) as pf, \
             tc.tile_pool(name="p_ffnt", bufs=2) as pft, \
             tc.tile_pool(name="p_wstream", bufs=3) as pws:
            xq_sb = load(pf, xqT, (P, CC, TQ), f32, tag="xq_res")
            wo_s = load(pf, wo, (P, CC, 8, P), bf)
            y1 = pf.tile([P, CC, TQ], f32)
            for mo in range(CC):
                psum = ppool.tile([P, 512], f32, tag="mm")
                for cc in range(CC):
                    nc.tensor.matmul(psum, lhsT=wo_s[:, cc, mo, :],
                                     rhs=OT[:, cc, :],
                                     start=(cc == 0), stop=(cc == CC - 1))
                t = pft.tile([P, TQ], f32, tag="res")
                nc.vector.tensor_scalar_add(t, psum, bo_s[:, mo:mo + 1])
                nc.vector.tensor_tensor(y1[:, mo, :], t, xq_sb[:, mo, :], ALU.add)

            h2 = pf.tile([P, CC, TQ], bf)
            with tc.tile_pool(name="p_ln2b", bufs=1) as pl2b:
                _ln_T(nc, pl2b, pft, spsum, y1, TQ, h2,
                      g2_s, be2_s, eps11, ones1, x_is_f32=True)

            zT = pf.tile([P, 32, TQ], bf)
            for m in range(32):
                w1b = pws.tile([P, CC, P], bf, tag="w1")
                nc.sync.dma_start(w1b, w1[m])
                psum = ppool.tile([P, 512], f32, tag="mm")
                for cc in range(CC):
                    nc.tensor.matmul(psum, lhsT=w1b[:, cc, :], rhs=h2[:, cc, :],
                                     start=(cc == 0), stop=(cc == CC - 1))
                nc.scalar.activation(zT[:, m, :], psum, AF.Relu,
                                     bias=b1_s[:, m:m + 1])

            for mo in range(CC):
                w2b = pws.tile([P, 32, P], bf, tag="w2")
                nc.sync.dma_start(w2b, w2[mo])
                psum = ppool.tile([P, 512], f32, tag="mm")
                for ff in range(32):
                    nc.tensor.matmul(psum, lhsT=w2b[:, ff, :], rhs=zT[:, ff, :],
                                     start=(ff == 0), stop=(ff == 31))
                t = pft.tile([P, TQ], f32, tag="res")
                nc.vector.tensor_scalar_add(t, psum, b2_s[:, mo:mo + 1])
                ot = pft.tile([P, TQ], f32, tag="ot")
                nc.vector.tensor_tensor(ot, t, y1[:, mo, :], ALU.add)
                nc.sync.dma_start(outT[:, mo, :], ot)


_NC_CACHE = {}
USE_AG = False


def build_nc(reps=1, use_ag=None):
    global _NC_CACHE
    if use_ag is None:
        use_ag = USE_AG
    key = (reps, use_ag)
    if key in _NC_CACHE:
        return _NC_CACHE[key]
    nc = bacc.Bacc("TRN2", target_bir_lowering=False, debug=False,
                   enable_asserts=False, num_devices=8)

    def dram(name, shape, dtype, kind="ExternalInput"):
        return nc.dram_tensor(name, shape, dtype, kind=kind).ap()

    aps = (
        dram("xkvT", (P, CC, T), bf) if not use_ag else None,
        dram("xqT", (P, CC, TQ), f32),
        dram("maskT", (P, NSCH, TQ), bf),
        dram("wq", (P, CC, 8, P), bf),
        dram("wk", (P, CC, 8, P), bf),
        dram("wv", (P, CC, C), bf),
        dram("wo", (P, CC, 8, P), bf),
        dram("w1", (32, P, CC, P), bf),
        dram("w2", (CC, P, 32, P), bf),
        dram("bo_t", (P, CC), f32),
        dram("b1_t", (P, 32), f32),
        dram("b2_t", (P, CC), f32),
        dram("g1_t", (P, CC), f32),
        dram("be1_t", (P, CC), f32),
        dram("g2_t", (P, CC), f32),
        dram("be2_t", (P, CC), f32),
        dram("outT", (P, CC, TQ), f32, kind="ExternalOutput"),
    )
    bounces = None
    if use_ag:
        bounces = []
        for i in range(reps):
            kv_in = nc.dram_tensor(f"kv_in{i}", (P, FKV), bf)
            kv_out = nc.dram_tensor(f"kv_out{i}", (4 * P, FKV), bf)
            bounces.append((kv_in, kv_out))
    with tile.TileContext(nc) as tc:
        for i in range(reps):
            _body(nc, tc, aps, use_ag, bounces[i] if use_ag else None)
    nc.compile()
    _NC_CACHE[key] = nc
    return nc


def _tile_lhst(w):  # (C, C) -> (P, cc, pair/mo, 128)
    return np.ascontiguousarray(
        w.reshape(CC, P, 8, P).transpose(1, 0, 2, 3)).astype(bf16)


def make_in_maps(inputs, use_ag=None):
    """Build the 8 per-core input dicts from the full problem inputs."""
    if use_ag is None:
        use_ag = USE_AG
    x = np.asarray(inputs["x"], np.float32)
    Wq = np.asarray(inputs["Wq"], np.float32)
    Wk = np.asarray(inputs["Wk"], np.float32)
    Wv = np.asarray(inputs["Wv"], np.float32)
    Wo = np.asarray(inputs["Wo"], np.float32)
    W1 = np.asarray(inputs["W1"], np.float32)
    W2 = np.asarray(inputs["W2"], np.float32)

    wq_flat = np.ascontiguousarray(Wq.transpose(1, 0, 2)).reshape(C, C)
    wk_flat = np.ascontiguousarray(Wk.transpose(1, 0, 2)).reshape(C, C)
    wv_flat = np.ascontiguousarray(Wv.transpose(1, 0, 2)).reshape(C, C)

    shared = {
        "wq": _tile_lhst(wq_flat),
        "wk": _tile_lhst(wk_flat),
        "wv": np.ascontiguousarray(
            wv_flat.reshape(CC, P, C).transpose(1, 0, 2)).astype(bf16),
        "wo": _tile_lhst(Wo),
        "w1": np.ascontiguousarray(
            W1.reshape(CC, P, 32, P).transpose(2, 1, 0, 3)).astype(bf16),
        "w2": np.ascontiguousarray(
            W2.reshape(32, P, CC, P).transpose(2, 1, 0, 3)).astype(bf16),
        "bo_t": np.ascontiguousarray(
            np.asarray(inputs["bo"], np.float32).reshape(CC, P).T),
        "b1_t": np.ascontiguousarray(
            np.asarray(inputs["b1"], np.float32).reshape(32, P).T),
        "b2_t": np.ascontiguousarray(
            np.asarray(inputs["b2"], np.float32).reshape(CC, P).T),
        "g1_t": np.ascontiguousarray(
            np.asarray(inputs["g1"], np.float32).reshape(CC, P).T),
        "be1_t": np.ascontiguousarray(
            np.asarray(inputs["be1"], np.float32).reshape(CC, P).T),
        "g2_t": np.ascontiguousarray(
            np.asarray(inputs["g2"], np.float32).reshape(CC, P).T),
        "be2_t": np.ascontiguousarray(
            np.asarray(inputs["be2"], np.float32).reshape(CC, P).T),
    }

    s_idx = np.arange(T)
    in_maps = []
    for c in range(8):
        b, a = c // 4, c % 4
        q0 = TQ * a
        xbT = np.ascontiguousarray(x[b].T)                       # (C, T)
        xkvT = xbT.reshape(CC, P, T).transpose(1, 0, 2).astype(bf16)
        xqT = np.ascontiguousarray(
            xbT[:, q0:q0 + TQ].reshape(CC, P, TQ).transpose(1, 0, 2))
        mask = np.where(s_idx[:, None] <= (q0 + np.arange(TQ))[None, :],
                        np.float32(0.0), np.float32(NEG))
        maskT = mask.reshape(NSCH, P, TQ).transpose(1, 0, 2).astype(bf16)
        m = {
            "xqT": xqT.astype(np.float32),
            "maskT": np.ascontiguousarray(maskT),
            **shared,
        }
        if not use_ag:
            m["xkvT"] = np.ascontiguousarray(xkvT)
        in_maps.append(m)
    return in_maps


def assemble_output(core_outs):
    """core_outs: list of 8 dicts with 'outT' (P, CC, TQ) fp32."""
    out = np.zeros((B, T, C), np.float32)
    for c in range(8):
        b, a = c // 4, c % 4
        y2 = core_outs[c]["outT"].transpose(1, 0, 2).reshape(C, TQ)  # (C, TQ)
        out[b, TQ * a:TQ * (a + 1), :] = y2.T
    return out


def kernel(**inputs) -> np.ndarray:
    nc = build_nc()
    in_maps = make_in_maps(inputs)
    res = bass_utils.run_bass_kernel_spmd(nc, in_maps, core_ids=list(range(8)))
    return assemble_output(res.results)


if __name__ == "__main__":
    import reference
    inputs = {k: np.asarray(v) for k, v in reference.setup_inputs().items()}
    expected = np.asarray(reference.reference(**inputs))
    actual = kernel(**inputs)
    err = np.abs(actual - expected)
    print("absmax err:", err.max(), "scale:", np.abs(expected).max())
    print("rel fro:", np.linalg.norm(actual - expected) / np.linalg.norm(expected))



# revision 2
# speedup vs baseline: 1.9424x; 1.9424x over previous
"""Trainium2 Bass kernel for a dense transformer block (LN -> 16-head causal
attention -> residual -> LN -> FFN -> residual) on x:(2, 2048, 1024) fp32.

Head-sharded design, 8 cores, one ReduceScatter:
  core c = (batch b=c//4, head-group g=c%4).  Each core:
    1. streams x[b] (full 2048 tokens), recomputes LN1, builds h (bf16).
    2. projects Q,K,V for ITS 4 heads over all 2048 tokens (no duplication
       across the machine; no collective).
    3. causal attention for its 4 heads: query chunks of 512 attend only to
       key chunks 0..4s+3 -- the causal triangle is identical on every core,
       so the SPMD program skips ~44%% of score/AV work with no per-core
       control flow.  Diagonal chunks get a shared additive mask slab added
       in-place in PSUM before exp.
    4. partial output projection (its 256 of 1024 contraction rows) for all
       tokens, then ONE ReduceScatter(add) over the 4-core batch group
       delivers the summed attention output for its own 512 tokens.
    5. residual + LN2 + FFN (fp8 DoubleRow matmuls) + residual for its own
       512 tokens; writes its slice of the output.

LayerNorm affine params are folded exactly: g1 into Wq/Wk/Wv rows, be1 via
projection biases; g2 into W1, be2 into the FFN1 bias.  FFN weights are
pre-scaled (x32 / x64) into fp8 range and descaled in the epilogues.
"""

import numpy as np
import ml_dtypes

import concourse.bass as bass
import concourse.tile as tile
from concourse import bacc, mybir
from concourse import bass_utils

P = 128
B, T, C = 2, 2048, 1024
H, D = 16, 64
FF = 4 * C
CC = C // P            # 8 feature chunks
TQ = 512               # own tokens per core
HPC = 4                # heads per core
NS = T // TQ           # 4 query superchunks
NT = T // P            # 16 key chunks
EPS = 1e-5
NEG = -30000.0
SCL = float(C) ** -0.5
W1S, W2S = 32.0, 64.0  # fp8 pre-scales

bf16 = ml_dtypes.bfloat16
e4m3 = ml_dtypes.float8_e4m3

f32 = mybir.dt.float32
f32r = mybir.dt.float32r
bf = mybir.dt.bfloat16
fp8 = mybir.dt.float8e4
AF = mybir.ActivationFunctionType
ALU = mybir.AluOpType
DR = mybir.MatmulPerfMode.DoubleRow

USE_FP8 = True
QKV_BIAS = False   # be1 is structurally zero in this problem's setup_inputs;
                   # kernel() switches to the biased variant if data says else
S_ORDER = (0, 1, 2, 3)

# the diagonal 128x128 causal triangle (shared by all cores/heads/chunks)
SLAB_TOT = 128


def _body(nc, tc, aps):
    (xT, x_own_d, wq, wk, wv, wo, w1d, w2d,
     bq_t, bk_t, bv_bc_d, bo_t, b1p_t, b2_t, slab_d,
     rs_in, rs_out, outT) = aps

    import contextlib
    ctx = contextlib.ExitStack()
    with ctx:
        ctx.enter_context(nc.allow_low_precision(
            reason="LN apply + softmax intermediates are bf16 by design"))
        consts = ctx.enter_context(tc.tile_pool(name="consts", bufs=1))
        ppool = ctx.enter_context(tc.tile_pool(name="ppool", bufs=5, space="PSUM"))
        opsum = ctx.enter_context(tc.tile_pool(name="opsum", bufs=2, space="PSUM"))
        spsum = ctx.enter_context(tc.tile_pool(name="spsum", bufs=1, space="PSUM"))
        small = ctx.enter_context(tc.tile_pool(name="small", bufs=2))
        lnx = ctx.enter_context(tc.tile_pool(name="lnx", bufs=2))

        def load(pool, ap_dram, shape, dtype=f32, tag=None):
            # consts go on the Act HWDGE queue so the phase-1 x-chunk
            # stream (SP queue) isn't stuck behind 1.2MB of weights
            t = pool.tile(list(shape), dtype, tag=tag or ap_dram.name)
            nc.scalar.dma_start(t, ap_dram)
            return t

        ones_f = consts.tile([P, 1], f32)
        nc.vector.memset(ones_f, 1.0)
        ones_b = consts.tile([P, 1], bf)
        nc.vector.memset(ones_b, 1.0)
        eps11 = consts.tile([1, 1], f32)
        nc.vector.memset(eps11, EPS)

        bq_s = load(consts, bq_t, (P, 2))
        bk_s = load(consts, bk_t, (P, 2))
        bv_bc = load(consts, bv_bc_d, (P, HPC * D))
        bo_s = load(consts, bo_t, (P, CC))
        b1_s = load(consts, b1p_t, (P, 32))
        b2_s = load(consts, b2_t, (P, CC))
        slab = load(consts, slab_d, (P, SLAB_TOT), bf)

        wq_s = load(consts, wq, (P, CC, 2, P), bf)
        wk_s = load(consts, wk, (P, CC, 2, P), bf)
        wv_s = load(consts, wv, (P, CC, HPC * D), bf)
        wo_s = load(consts, wo, (P, 2, CC, P), bf)

        # ---- long-lived activations
        kvq_ctx = contextlib.ExitStack()
        kvq = kvq_ctx.enter_context(tc.tile_pool(name="kvq", bufs=1))
        QT = kvq.tile([P, 2, T], bf)
        KT = kvq.tile([P, 2, T], bf)
        Vr = kvq.tile([P, NT, HPC, 65], bf)
        nc.vector.memset(Vr[:, :, :, 64:65], 1.0)

        ot_ctx = contextlib.ExitStack()
        otp = ot_ctx.enter_context(tc.tile_pool(name="otp", bufs=1))
        OT = otp.tile([P, 2, T], bf)

        def ln_stats(xs, n, lnt, tag):
            """xs: (P, CC, n) f32 in SBUF -> A_bc, B_bc (P, n) f32.
            Stats computed from a bf16 copy (Act engine) like the reference
            bf16 matmul path; walrus rejects f32r-bitcast matmuls."""
            # both stat rows share one psum bank (partitions 0 and 32)
            pspq = spsum.tile([33, n], f32, tag="pspq")
            ps = pspq[0:1, :]
            pq = pspq[32:33, :]
            xbs = []
            for cc in range(CC):
                xb = lnx.tile([P, n], bf, tag=f"xb{cc}")
                nc.scalar.activation(xb, xs[:, cc, :], AF.Copy)
                xbs.append(xb)
                sq = lnx.tile([P, n], bf, tag="sq")
                nc.vector.tensor_mul(sq, xb, xb)
                nc.tensor.matmul(ps, lhsT=ones_b, rhs=xb,
                                 start=(cc == 0), stop=(cc == CC - 1))
                nc.tensor.matmul(pq, lhsT=ones_b, rhs=sq,
                                 start=(cc == 0), stop=(cc == CC - 1))
            m = small.tile([1, n], f32, tag=f"m{tag}")
            nc.vector.tensor_scalar_mul(m, ps, 1.0 / C)
            q = small.tile([1, n], f32, tag=f"q{tag}")
            nc.vector.tensor_scalar_mul(q, pq, 1.0 / C)
            msq = small.tile([1, n], f32, tag=f"msq{tag}")
            nc.vector.tensor_mul(msq, m, m)
            nc.vector.tensor_tensor(q, q, msq, ALU.subtract)  # q := var
            sd = small.tile([1, n], f32, tag=f"sd{tag}")
            nc.scalar.activation(sd, q, AF.Sqrt, bias=eps11)
            A = small.tile([1, n], bf, tag=f"A{tag}")
            nc.vector.reciprocal(A, sd)
            Bm = small.tile([1, n], bf, tag=f"B{tag}")
            nc.vector.tensor_mul(Bm, m, A)
            A_bc = lnt.tile([P, n], bf, tag=f"Abc{tag}")
            nc.gpsimd.partition_broadcast(A_bc, A)
            B_bc = lnt.tile([P, n], bf, tag=f"Bbc{tag}")
            nc.gpsimd.partition_broadcast(B_bc, Bm)
            return A_bc, B_bc, xbs

        # ================= phase 1+2: LN1, h, Q/K/V  =================
        with tc.tile_pool(name="hpool", bufs=1) as hpool, \
             tc.tile_pool(name="xs_p", bufs=2) as xsp, \
             tc.tile_pool(name="lnt", bufs=3) as lnt:
            h = hpool.tile([P, CC, T], bf)
            for s in range(NS):
                xs = xsp.tile([P, CC, TQ], f32, tag="xs")
                nc.sync.dma_start(xs, xT[:, :, bass.ts(s, TQ)])
                A_bc, B_bc, xbs = ln_stats(xs, TQ, lnt, "1")
                for cc in range(CC):
                    tt = lnt.tile([P, TQ], bf, tag="app")
                    nc.vector.tensor_mul(tt, xbs[cc], A_bc)
                    nc.vector.tensor_tensor(h[:, cc, bass.ts(s, TQ)],
                                            tt, B_bc, ALU.subtract)
                hs = h[:, :, bass.ts(s, TQ)]
                for pair in range(2):
                    psq = ppool.tile([P, TQ], f32, tag="mm")
                    for cc in range(CC):
                        nc.tensor.matmul(psq, lhsT=wq_s[:, cc, pair, :],
                                         rhs=hs[:, cc, :],
                                         start=(cc == 0), stop=(cc == CC - 1))
                    if QKV_BIAS:
                        nc.vector.tensor_scalar(QT[:, pair, bass.ts(s, TQ)],
                                                psq, scalar1=SCL,
                                                scalar2=bq_s[:, pair:pair + 1],
                                                op0=ALU.mult, op1=ALU.add)
                    else:
                        nc.scalar.activation(QT[:, pair, bass.ts(s, TQ)],
                                             psq, AF.Copy, scale=SCL)
                    psk = ppool.tile([P, TQ], f32, tag="mm")
                    for cc in range(CC):
                        nc.tensor.matmul(psk, lhsT=wk_s[:, cc, pair, :],
                                         rhs=hs[:, cc, :],
                                         start=(cc == 0), stop=(cc == CC - 1))
                    if QKV_BIAS:
                        nc.vector.tensor_scalar_add(
                            KT[:, pair, bass.ts(s, TQ)], psk,
                            bk_s[:, pair:pair + 1])
                    else:
                        nc.scalar.activation(KT[:, pair, bass.ts(s, TQ)],
                                             psk, AF.Copy)
                for jj in range(4):
                    j = 4 * s + jj
                    psvt = ppool.tile([P, TQ], f32, tag="mm")
                    psv = psvt[:, 0:HPC * D]
                    for cc in range(CC):
                        nc.tensor.matmul(psv, lhsT=hs[:, cc, bass.ts(jj, P)],
                                         rhs=wv_s[:, cc, :],
                                         start=(cc == 0), stop=(cc == CC - 1))
                    if QKV_BIAS:
                        nc.vector.tensor_tensor(
                            Vr[:, j, :, 0:64],
                            psv.rearrange("p (h d) -> p h d", d=D),
                            bv_bc.rearrange("p (h d) -> p h d", d=D), ALU.add)
                    else:
                        nc.scalar.activation(
                            Vr[:, j, :, 0:64],
                            psv.rearrange("p (h d) -> p h d", d=D), AF.Copy)

        # ================= phase 3: attention + out-proj =================
        with tc.tile_pool(name="epool", bufs=3) as epool, \
             tc.tile_pool(name="zpool", bufs=2) as zpool, \
             tc.tile_pool(name="rsst", bufs=2) as rsst:
            for s in S_ORDER:
                # start with a superchunk that only needs early K/V (overlaps
                # phase 2), end with a small one (short tail before the RS)
                nv = 4 * s + 4
                for hh0 in (0, 2):
                    # two heads interleaved: fills the scores->exp->AV latency
                    O0 = opsum.tile([P, TQ], f32, tag="av")
                    O1 = opsum.tile([P, TQ], f32, tag="av")
                    Os = [O0, O1]
                    for j in range(nv):
                        for u in range(2):
                            hh = hh0 + u
                            pair, half = hh // 2, hh % 2
                            hp = slice(64 * half, 64 * half + 64)
                            S = ppool.tile([P, TQ], f32, tag="mm")
                            nc.tensor.matmul(S,
                                             lhsT=KT[hp, pair, bass.ts(j, P)],
                                             rhs=QT[hp, pair, bass.ts(s, TQ)],
                                             start=True, stop=True)
                            e = epool.tile([P, TQ], bf, tag="e")
                            if j >= 4 * s:
                                # diagonal chunk d: queries < 128d see none of
                                # these keys (e=0); queries in [128d,128d+128)
                                # need the triangular mask; rest fully visible.
                                d = j - 4 * s
                                z0 = 128 * d
                                nc.vector.tensor_tensor(
                                    S[:, z0:z0 + P], S[:, z0:z0 + P],
                                    slab, ALU.add)
                                if z0:
                                    nc.gpsimd.memset(e[:, 0:z0], 0.0)
                                nc.scalar.activation(e[:, z0:], S[:, z0:],
                                                     AF.Exp)
                            else:
                                nc.scalar.activation(e, S, AF.Exp)
                            nc.tensor.matmul(Os[u][0:65, :],
                                             lhsT=Vr[:, j, hh, :],
                                             rhs=e, start=(j == 0),
                                             stop=(j == nv - 1))
                    for u in range(2):
                        hh = hh0 + u
                        pair, half = hh // 2, hh % 2
                        hp = slice(64 * half, 64 * half + 64)
                        zr = small.tile([1, TQ], f32, tag="zr")
                        nc.vector.reciprocal(zr, Os[u][64:65, :])
                        zb = zpool.tile([64, TQ], f32, tag="zb")
                        nc.gpsimd.partition_broadcast(zb, zr)
                        nc.vector.tensor_mul(OT[hp, pair, bass.ts(s, TQ)],
                                             Os[u][0:64, :], zb)
                stg = rsst.tile([P, CC, TQ], bf, tag="stg")
                for mo in range(CC):
                    pso = ppool.tile([P, TQ], f32, tag="mm")
                    for kk in range(2):
                        nc.tensor.matmul(pso, lhsT=wo_s[:, kk, mo, :],
                                         rhs=OT[:, kk, bass.ts(s, TQ)],
                                         start=(kk == 0), stop=(kk == 1))
                    nc.vector.tensor_copy(stg[:, mo, :], pso)
                nc.sync.dma_start(rs_in[bass.ts(s, P), :, :], stg)
        ot_ctx.close()
        kvq_ctx.close()

        nc.gpsimd.collective_compute(
            "ReduceScatter", ALU.add,
            replica_groups=[[0, 1, 2, 3], [4, 5, 6, 7]],
            ins=[rs_in.opt()], outs=[rs_out.opt()])

        # ================= phase 4: residual, LN2, FFN =================
        with tc.tile_pool(name="ffp", bufs=1) as ffp, \
             tc.tile_pool(name="lnt2", bufs=2) as lnt2, \
             tc.tile_pool(name="w1p", bufs=3) as w1p, \
             tc.tile_pool(name="w2p", bufs=2) as w2p, \
             tc.tile_pool(name="fft", bufs=2) as fft:
            x_own = ffp.tile([P, CC, TQ], f32)
            nc.sync.dma_start(x_own, x_own_d)
            if USE_FP8:
                # whole W1 (4MB fp8) lands during the ReduceScatter
                w1full = ffp.tile([P, 32, 4, 2, P], fp8)
                nc.sync.dma_start(w1full, w1d)
            y1 = ffp.tile([P, CC, TQ], f32)
            rs_sb = ffp.tile([P, CC, TQ], bf)
            nc.sync.dma_start(rs_sb, rs_out)
            for mo in range(CC):
                nc.vector.scalar_tensor_tensor(y1[:, mo, :], rs_sb[:, mo, :],
                                               bo_s[:, mo:mo + 1],
                                               x_own[:, mo, :],
                                               ALU.add, ALU.add)
            A2, B2, ybs = ln_stats(y1, TQ, lnt2, "2")
            if USE_FP8:
                h2 = ffp.tile([P, 4, 2, TQ], fp8)
                z = ffp.tile([P, 16, 2, TQ], fp8)
                for cc in range(CC):
                    tt = lnt2.tile([P, TQ], bf, tag="app2")
                    nc.vector.tensor_mul(tt, ybs[cc], A2)
                    nc.vector.tensor_tensor(h2[:, cc // 2, cc % 2, :],
                                            tt, B2, ALU.subtract)
                for m in range(32):
                    psf = ppool.tile([P, TQ], f32, tag="mm")
                    for k4 in range(4):
                        nc.tensor.matmul(psf, lhsT=w1full[:, m, k4, :, :],
                                         rhs=h2[:, k4, :, :],
                                         start=(k4 == 0), stop=(k4 == 3),
                                         perf_mode=DR)
                    nc.scalar.activation(z[:, m // 2, m % 2, :], psf, AF.Relu,
                                         bias=b1_s[:, m:m + 1],
                                         scale=1.0 / W1S)
                # FFN2 in two half-contraction passes: the k16<8 pass only
                # needs z from FFN1's first 16 m-tiles, so it overlaps
                # FFN1's second half.
                ffhalf = ffp.tile([P, CC, TQ], bf)
                for mo in range(CC):
                    w2t = w2p.tile([P, 8, 2, P], fp8, tag="w2a")
                    nc.sync.dma_start(w2t, w2d[mo, :, 0:8])
                    psa = opsum.tile([P, TQ], f32, tag="av")
                    for k16 in range(8):
                        nc.tensor.matmul(psa, lhsT=w2t[:, k16, :, :],
                                         rhs=z[:, k16, :, :],
                                         start=(k16 == 0), stop=(k16 == 7),
                                         perf_mode=DR)
                    nc.vector.tensor_scalar(ffhalf[:, mo, :], psa,
                                            scalar1=1.0 / W2S,
                                            scalar2=b2_s[:, mo:mo + 1],
                                            op0=ALU.mult, op1=ALU.add)
                for mo in range(CC):
                    w2t = w2p.tile([P, 8, 2, P], fp8, tag="w2b")
                    nc.sync.dma_start(w2t, w2d[mo, :, 8:16])
                    psf = ppool.tile([P, TQ], f32, tag="mm")
                    for k16 in range(8):
                        nc.tensor.matmul(psf, lhsT=w2t[:, k16, :, :],
                                         rhs=z[:, 8 + k16, :, :],
                                         start=(k16 == 0), stop=(k16 == 7),
                                         perf_mode=DR)
                    tt = fft.tile([P, TQ], f32, tag="ep2")
                    nc.vector.tensor_scalar_mul(tt, psf, 1.0 / W2S)
                    acc = fft.tile([P, TQ], f32, tag="acc")
                    nc.vector.tensor_tensor(acc, tt, ffhalf[:, mo, :], ALU.add)
                    ot = fft.tile([P, TQ], f32, tag="ot")
                    nc.vector.tensor_tensor(ot, acc, y1[:, mo, :], ALU.add)
                    nc.sync.dma_start(outT[:, mo, :], ot)
            else:
                h2 = ffp.tile([P, CC, TQ], bf)
                z = ffp.tile([P, 32, TQ], bf)
                for cc in range(CC):
                    tt = lnt2.tile([P, TQ], bf, tag="app2")
                    nc.vector.tensor_mul(tt, ybs[cc], A2)
                    nc.vector.tensor_tensor(h2[:, cc, :], tt, B2,
                                            ALU.subtract)
                for m in range(32):
                    w1t = w1p.tile([P, CC, P], bf, tag="w1")
                    nc.sync.dma_start(w1t, w1d[m])
                    psf = ppool.tile([P, TQ], f32, tag="mm")
                    for cc in range(CC):
                        nc.tensor.matmul(psf, lhsT=w1t[:, cc, :],
                                         rhs=h2[:, cc, :],
                                         start=(cc == 0), stop=(cc == CC - 1))
                    nc.scalar.activation(z[:, m, :], psf, AF.Relu,
                                         bias=b1_s[:, m:m + 1])
                for mo in range(CC):
                    w2t = w2p.tile([P, 32, P], bf, tag="w2")
                    nc.sync.dma_start(w2t, w2d[mo])
                    psf = ppool.tile([P, TQ], f32, tag="mm")
                    for ff in range(32):
                        nc.tensor.matmul(psf, lhsT=w2t[:, ff, :],
                                         rhs=z[:, ff, :],
                                         start=(ff == 0), stop=(ff == 31))
                    tt = fft.tile([P, TQ], f32, tag="ep")
                    nc.vector.tensor_scalar_add(tt, psf, b2_s[:, mo:mo + 1])
                    ot = fft.tile([P, TQ], f32, tag="ot")
                    nc.vector.tensor_tensor(ot, tt, y1[:, mo, :], ALU.add)
                    nc.sync.dma_start(outT[:, mo, :], ot)


_NC_CACHE = {}


def build_nc(reps=1, use_fp8=None, qkv_bias=None):
    global USE_FP8, QKV_BIAS
    if use_fp8 is not None:
        USE_FP8 = use_fp8
    if qkv_bias is not None:
        QKV_BIAS = qkv_bias
    key = (reps, USE_FP8, QKV_BIAS)
    if key in _NC_CACHE:
        return _NC_CACHE[key]
    nc = bacc.Bacc("TRN2", target_bir_lowering=False, debug=False,
                   enable_asserts=False, num_devices=8)

    def dram(name, shape, dtype, kind="ExternalInput"):
        return nc.dram_tensor(name, shape, dtype, kind=kind).ap()

    if USE_FP8:
        w1_shape, w2_shape, wdt = (P, 32, 4, 2, P), (CC, P, 16, 2, P), fp8
    else:
        w1_shape, w2_shape, wdt = (32, P, CC, P), (CC, P, 32, P), bf

    aps = (
        dram("xT", (P, CC, T), f32),
        dram("x_own", (P, CC, TQ), f32),
        dram("wq", (P, CC, 2, P), bf),
        dram("wk", (P, CC, 2, P), bf),
        dram("wv", (P, CC, HPC * D), bf),
        dram("wo", (P, 2, CC, P), bf),
        dram("w1", w1_shape, wdt),
        dram("w2", w2_shape, wdt),
        dram("bq_t", (P, 2), f32),
        dram("bk_t", (P, 2), f32),
        dram("bv_bc", (P, HPC * D), f32),
        dram("bo_t", (P, CC), f32),
        dram("b1p_t", (P, 32), f32),
        dram("b2_t", (P, CC), f32),
        dram("slab", (P, SLAB_TOT), bf),
        nc.dram_tensor("rs_in", (4 * P, CC, TQ), bf).ap(),
        nc.dram_tensor("rs_out", (P, CC, TQ), bf).ap(),
        dram("outT", (P, CC, TQ), f32, kind="ExternalOutput"),
    )
    with tile.TileContext(nc) as tc:
        for _ in range(reps):
            _body(nc, tc, aps)
    nc.compile()
    _NC_CACHE[key] = nc
    return nc


def make_in_maps(inputs, use_fp8=None):
    if use_fp8 is None:
        use_fp8 = USE_FP8
    x = np.asarray(inputs["x"], np.float32)
    Wq = np.asarray(inputs["Wq"], np.float32)
    Wk = np.asarray(inputs["Wk"], np.float32)
    Wv = np.asarray(inputs["Wv"], np.float32)
    Wo = np.asarray(inputs["Wo"], np.float32)
    bo = np.asarray(inputs["bo"], np.float32)
    W1 = np.asarray(inputs["W1"], np.float32)
    b1 = np.asarray(inputs["b1"], np.float32)
    W2 = np.asarray(inputs["W2"], np.float32)
    b2 = np.asarray(inputs["b2"], np.float32)
    g1 = np.asarray(inputs["g1"], np.float32)
    be1 = np.asarray(inputs["be1"], np.float32)
    g2 = np.asarray(inputs["g2"], np.float32)
    be2 = np.asarray(inputs["be2"], np.float32)

    # fold g1 into QKV weight rows; be1 into projection biases
    Wqg = Wq * g1[None, :, None]          # (H, C, D)
    Wkg = Wk * g1[None, :, None]
    Wvg = Wv * g1[None, :, None]
    bq_h = np.einsum("c,hcd->hd", be1, Wq)   # (H, D)
    bk_h = np.einsum("c,hcd->hd", be1, Wk)
    bv_h = np.einsum("c,hcd->hd", be1, Wv)
    # fold g2 into W1 rows; be2 into b1
    W1g = W1 * g2[:, None]
    b1p = b1 + be2 @ W1                      # (FF,)

    # fp8/bf16 FFN weights
    if use_fp8:
        w1_host = np.ascontiguousarray(
            (W1S * W1g).reshape(4, 2, P, 32, P).transpose(2, 3, 0, 1, 4)
        ).astype(e4m3)                       # (P, 32, 4, 2, P)
        w2_host = np.ascontiguousarray(
            (W2S * W2).reshape(16, 2, P, CC, P).transpose(2, 0, 1, 3, 4))
        w2_host = np.ascontiguousarray(
            w2_host.transpose(3, 0, 1, 2, 4)).astype(e4m3)  # (8, P, 16, 2, P)
    else:
        w1_host = np.ascontiguousarray(
            W1g.reshape(CC, P, 32, P).transpose(2, 1, 0, 3)).astype(bf16)
        w2_host = np.ascontiguousarray(
            W2.reshape(32, P, CC, P).transpose(2, 1, 0, 3)).astype(bf16)

    # shared diagonal mask slab
    k = np.arange(P)[:, None]
    q = np.arange(P)[None, :]
    slab = np.where(k <= q, 0.0, NEG).astype(bf16)

    b2t = np.ascontiguousarray(b2.reshape(CC, P).T)
    bot = np.ascontiguousarray(bo.reshape(CC, P).T)
    b1t = np.ascontiguousarray(b1p.reshape(32, P).T)

    in_maps = []
    for c in range(8):
        b, g = c // 4, c % 4
        hs = [4 * g + i for i in range(HPC)]
        xbT = np.ascontiguousarray(x[b].T)       # (C, T)
        xT_h = np.ascontiguousarray(
            xbT.reshape(CC, P, T).transpose(1, 0, 2))
        x_own_h = np.ascontiguousarray(xT_h[:, :, TQ * g:TQ * (g + 1)])

        # wq/wk: (P, CC, pair, 128): cols = [head 2pair | head 2pair+1]
        def qk_tile(Wg):
            out = np.zeros((P, CC, 2, P), np.float32)
            for pair in range(2):
                blk = np.concatenate(
                    [Wg[hs[2 * pair]], Wg[hs[2 * pair + 1]]], axis=1)  # (C,128)
                out[:, :, pair, :] = blk.reshape(CC, P, P).transpose(1, 0, 2)
            return out.astype(bf16)

        wv_h = np.concatenate([Wvg[h] for h in hs], axis=1)   # (C, 256)
        wv_host = np.ascontiguousarray(
            wv_h.reshape(CC, P, HPC * D).transpose(1, 0, 2)).astype(bf16)

        # wo: (P, kk, mo, 128): rows 256g..256g+255 of Wo
        wo_rows = Wo[256 * g:256 * (g + 1), :]                # (256, C)
        wo_host = np.ascontiguousarray(
            wo_rows.reshape(2, P, CC, P).transpose(1, 0, 2, 3)).astype(bf16)

        bq_pair = np.concatenate(
            [np.concatenate([bq_h[hs[2 * p]], bq_h[hs[2 * p + 1]]])[:, None]
             for p in range(2)], axis=1)                      # (128, 2)
        bk_pair = np.concatenate(
            [np.concatenate([bk_h[hs[2 * p]], bk_h[hs[2 * p + 1]]])[:, None]
             for p in range(2)], axis=1)
        bv_row = np.concatenate([bv_h[h] for h in hs])        # (256,)
        bv_bc_host = np.tile(bv_row[None, :], (P, 1)).astype(np.float32)

        in_maps.append({
            "xT": xT_h,
            "x_own": x_own_h,
            "wq": qk_tile(Wqg),
            "wk": qk_tile(Wkg),
            "wv": wv_host,
            "wo": wo_host,
            "w1": w1_host,
            "w2": w2_host,
            "bq_t": np.ascontiguousarray(bq_pair * SCL),
            "bk_t": np.ascontiguousarray(bk_pair),
            "bv_bc": bv_bc_host,
            "bo_t": bot,
            "b1p_t": b1t,
            "b2_t": b2t,
            "slab": slab,
        })
    return in_maps


def assemble_output(core_outs):
    out = np.zeros((B, T, C), np.float32)
    for c in range(8):
        b, g = c // 4, c % 4
        y2 = core_outs[c]["outT"].transpose(1, 0, 2).reshape(C, TQ)
        out[b, TQ * g:TQ * (g + 1), :] = y2.T
    return out


def kernel(**inputs) -> np.ndarray:
    in_maps = make_in_maps(inputs)
    need_bias = any(
        float(np.abs(m[k]).max()) > 0.0
        for m in in_maps[:1] for k in ("bq_t", "bk_t", "bv_bc"))
    nc = build_nc(qkv_bias=need_bias)
    res = bass_utils.run_bass_kernel_spmd(nc, in_maps, core_ids=list(range(8)))
    return assemble_output(res.results)


# revision 3
# speedup vs baseline: 1.9567x; 1.0073x over previous
"""Trainium2 Bass kernel for a dense transformer block (LN -> 16-head causal
attention -> residual -> LN -> FFN -> residual) on x:(2, 2048, 1024) fp32.

Head-sharded design, 8 cores, one ReduceScatter:
  core c = (batch b=c//4, head-group g=c%4).  Each core:
    1. streams x[b] (full 2048 tokens), recomputes LN1, builds h (bf16).
    2. projects Q,K,V for ITS 4 heads over all 2048 tokens (no duplication
       across the machine; no collective).
    3. causal attention for its 4 heads: query chunks of 512 attend only to
       key chunks 0..4s+3 -- the causal triangle is identical on every core,
       so the SPMD program skips ~44%% of score/AV work with no per-core
       control flow.  Diagonal chunks get a shared additive mask slab added
       in-place in PSUM before exp.
    4. partial output projection (its 256 of 1024 contraction rows) for all
       tokens, then ONE ReduceScatter(add) over the 4-core batch group
       delivers the summed attention output for its own 512 tokens.
    5. residual + LN2 + FFN (fp8 DoubleRow matmuls) + residual for its own
       512 tokens; writes its slice of the output.

LayerNorm affine params are folded exactly: g1 into Wq/Wk/Wv rows, be1 via
projection biases; g2 into W1, be2 into the FFN1 bias.  FFN weights are
pre-scaled (x32 / x64) into fp8 range and descaled in the epilogues.
"""

import numpy as np
import ml_dtypes

import concourse.bass as bass
import concourse.tile as tile
from concourse import bacc, mybir
from concourse import bass_utils

P = 128
B, T, C = 2, 2048, 1024
H, D = 16, 64
FF = 4 * C
CC = C // P            # 8 feature chunks
TQ = 512               # own tokens per core
HPC = 4                # heads per core
NS = T // TQ           # 4 query superchunks
NT = T // P            # 16 key chunks
EPS = 1e-5
NEG = -30000.0
SCL = float(C) ** -0.5
W1S, W2S = 32.0, 64.0  # fp8 pre-scales

bf16 = ml_dtypes.bfloat16
e4m3 = ml_dtypes.float8_e4m3

f32 = mybir.dt.float32
f32r = mybir.dt.float32r
bf = mybir.dt.bfloat16
fp8 = mybir.dt.float8e4
AF = mybir.ActivationFunctionType
ALU = mybir.AluOpType
DR = mybir.MatmulPerfMode.DoubleRow

USE_FP8 = True
QKV_BIAS = False   # be1 is structurally zero in this problem's setup_inputs;
                   # kernel() switches to the biased variant if data says else
S_ORDER = (0, 1, 2, 3)

# the diagonal 128x128 causal triangle (shared by all cores/heads/chunks)
SLAB_TOT = 128


def _body(nc, tc, aps):
    (xT, x_own_d, wq, wk, wv, wo, w1d, w2d,
     bq_t, bk_t, bv_bc_d, bo_t, b1p_t, b2_t, slab_d,
     rs_in, rs_out, outT) = aps

    import contextlib
    ctx = contextlib.ExitStack()
    with ctx:
        ctx.enter_context(nc.allow_low_precision(
            reason="LN apply + softmax intermediates are bf16 by design"))
        consts = ctx.enter_context(tc.tile_pool(name="consts", bufs=1))
        ppool = ctx.enter_context(tc.tile_pool(name="ppool", bufs=4, space="PSUM"))
        opsum = ctx.enter_context(tc.tile_pool(name="opsum", bufs=3, space="PSUM"))
        spsum = ctx.enter_context(tc.tile_pool(name="spsum", bufs=1, space="PSUM"))
        small = ctx.enter_context(tc.tile_pool(name="small", bufs=2))
        lnx = ctx.enter_context(tc.tile_pool(name="lnx", bufs=2))

        def load(pool, ap_dram, shape, dtype=f32, tag=None):
            # consts go on the Act HWDGE queue so the phase-1 x-chunk
            # stream (SP queue) isn't stuck behind 1.2MB of weights
            t = pool.tile(list(shape), dtype, tag=tag or ap_dram.name)
            nc.scalar.dma_start(t, ap_dram)
            return t

        ones_f = consts.tile([P, 1], f32)
        nc.vector.memset(ones_f, 1.0)
        ones_b = consts.tile([P, 1], bf)
        nc.vector.memset(ones_b, 1.0)
        eps11 = consts.tile([1, 1], f32)
        nc.vector.memset(eps11, EPS)

        bq_s = load(consts, bq_t, (P, 2))
        bk_s = load(consts, bk_t, (P, 2))
        bv_bc = load(consts, bv_bc_d, (P, HPC * D))
        bo_s = load(consts, bo_t, (P, CC))
        b1_s = load(consts, b1p_t, (P, 32))
        b2_s = load(consts, b2_t, (P, CC))
        slab = load(consts, slab_d, (P, SLAB_TOT), bf)

        wq_s = load(consts, wq, (P, CC, 2, P), bf)
        wk_s = load(consts, wk, (P, CC, 2, P), bf)
        wv_s = load(consts, wv, (P, CC, HPC * D), bf)
        wo_s = load(consts, wo, (P, 2, CC, P), bf)

        # ---- long-lived activations
        kvq_ctx = contextlib.ExitStack()
        kvq = kvq_ctx.enter_context(tc.tile_pool(name="kvq", bufs=1))
        QT = kvq.tile([P, 2, T], bf)
        KT = kvq.tile([P, 2, T], bf)
        Vr = kvq.tile([P, NT, HPC, 65], bf)
        nc.vector.memset(Vr[:, :, :, 64:65], 1.0)

        ot_ctx = contextlib.ExitStack()
        otp = ot_ctx.enter_context(tc.tile_pool(name="otp", bufs=1))
        OT = otp.tile([P, 2, T], bf)

        def ln_stats(xs, n, lnt, tag):
            """xs: (P, CC, n) f32 in SBUF -> A_bc, B_bc (P, n) f32.
            Stats computed from a bf16 copy (Act engine) like the reference
            bf16 matmul path; walrus rejects f32r-bitcast matmuls."""
            # both stat rows share one psum bank (partitions 0 and 32)
            pspq = spsum.tile([33, n], f32, tag="pspq")
            ps = pspq[0:1, :]
            pq = pspq[32:33, :]
            xbs = []
            for cc in range(CC):
                xb = lnx.tile([P, n], bf, tag=f"xb{cc}")
                nc.scalar.activation(xb, xs[:, cc, :], AF.Copy)
                xbs.append(xb)
                sq = lnx.tile([P, n], bf, tag="sq")
                nc.vector.tensor_mul(sq, xb, xb)
                nc.tensor.matmul(ps, lhsT=ones_b, rhs=xb,
                                 start=(cc == 0), stop=(cc == CC - 1))
                nc.tensor.matmul(pq, lhsT=ones_b, rhs=sq,
                                 start=(cc == 0), stop=(cc == CC - 1))
            m = small.tile([1, n], f32, tag=f"m{tag}")
            nc.vector.tensor_scalar_mul(m, ps, 1.0 / C)
            q = small.tile([1, n], f32, tag=f"q{tag}")
            nc.vector.tensor_scalar_mul(q, pq, 1.0 / C)
            msq = small.tile([1, n], f32, tag=f"msq{tag}")
            nc.vector.tensor_mul(msq, m, m)
            nc.vector.tensor_tensor(q, q, msq, ALU.subtract)  # q := var
            sd = small.tile([1, n], f32, tag=f"sd{tag}")
            nc.scalar.activation(sd, q, AF.Sqrt, bias=eps11)
            A = small.tile([1, n], bf, tag=f"A{tag}")
            nc.vector.reciprocal(A, sd)
            Bm = small.tile([1, n], bf, tag=f"B{tag}")
            nc.vector.tensor_mul(Bm, m, A)
            A_bc = lnt.tile([P, n], bf, tag=f"Abc{tag}")
            nc.gpsimd.partition_broadcast(A_bc, A)
            B_bc = lnt.tile([P, n], bf, tag=f"Bbc{tag}")
            nc.gpsimd.partition_broadcast(B_bc, Bm)
            return A_bc, B_bc, xbs

        # ================= phase 1+2: LN1, h, Q/K/V  =================
        with tc.tile_pool(name="hpool", bufs=1) as hpool, \
             tc.tile_pool(name="xs_p", bufs=2) as xsp, \
             tc.tile_pool(name="lnt", bufs=3) as lnt:
            h = hpool.tile([P, CC, T], bf)
            for s in range(NS):
                xs = xsp.tile([P, CC, TQ], f32, tag="xs")
                # two half-DMAs: stats on cc 0-3 start before cc 4-7 land
                nc.sync.dma_start(xs[:, 0:4, :], xT[:, 0:4, bass.ts(s, TQ)])
                nc.sync.dma_start(xs[:, 4:8, :], xT[:, 4:8, bass.ts(s, TQ)])
                A_bc, B_bc, xbs = ln_stats(xs, TQ, lnt, "1")
                for cc in range(CC):
                    tt = lnt.tile([P, TQ], bf, tag="app")
                    nc.vector.tensor_mul(tt, xbs[cc], A_bc)
                    nc.vector.tensor_tensor(h[:, cc, bass.ts(s, TQ)],
                                            tt, B_bc, ALU.subtract)
                hs = h[:, :, bass.ts(s, TQ)]
                for pair in range(2):
                    psq = ppool.tile([P, TQ], f32, tag="mm")
                    for cc in range(CC):
                        nc.tensor.matmul(psq, lhsT=wq_s[:, cc, pair, :],
                                         rhs=hs[:, cc, :],
                                         start=(cc == 0), stop=(cc == CC - 1))
                    if QKV_BIAS:
                        nc.vector.tensor_scalar(QT[:, pair, bass.ts(s, TQ)],
                                                psq, scalar1=SCL,
                                                scalar2=bq_s[:, pair:pair + 1],
                                                op0=ALU.mult, op1=ALU.add)
                    else:
                        nc.scalar.activation(QT[:, pair, bass.ts(s, TQ)],
                                             psq, AF.Copy, scale=SCL)
                    psk = ppool.tile([P, TQ], f32, tag="mm")
                    for cc in range(CC):
                        nc.tensor.matmul(psk, lhsT=wk_s[:, cc, pair, :],
                                         rhs=hs[:, cc, :],
                                         start=(cc == 0), stop=(cc == CC - 1))
                    if QKV_BIAS:
                        nc.vector.tensor_scalar_add(
                            KT[:, pair, bass.ts(s, TQ)], psk,
                            bk_s[:, pair:pair + 1])
                    else:
                        nc.scalar.activation(KT[:, pair, bass.ts(s, TQ)],
                                             psk, AF.Copy)
                for jj in range(4):
                    j = 4 * s + jj
                    psvt = ppool.tile([P, TQ], f32, tag="mm")
                    psv = psvt[:, 0:HPC * D]
                    for cc in range(CC):
                        nc.tensor.matmul(psv, lhsT=hs[:, cc, bass.ts(jj, P)],
                                         rhs=wv_s[:, cc, :],
                                         start=(cc == 0), stop=(cc == CC - 1))
                    if QKV_BIAS:
                        nc.vector.tensor_tensor(
                            Vr[:, j, :, 0:64],
                            psv.rearrange("p (h d) -> p h d", d=D),
                            bv_bc.rearrange("p (h d) -> p h d", d=D), ALU.add)
                    else:
                        nc.scalar.activation(
                            Vr[:, j, :, 0:64],
                            psv.rearrange("p (h d) -> p h d", d=D), AF.Copy)

        # ================= phase 3: attention + out-proj =================
        with tc.tile_pool(name="epool", bufs=3) as epool, \
             tc.tile_pool(name="zpool", bufs=2) as zpool, \
             tc.tile_pool(name="rsst", bufs=2) as rsst:
            for s in S_ORDER:
                # start with a superchunk that only needs early K/V (overlaps
                # phase 2), end with a small one (short tail before the RS)
                nv = 4 * s + 4
                for hh0 in (0, 2):
                    # two heads interleaved: fills the scores->exp->AV latency
                    O0 = opsum.tile([P, TQ], f32, tag="av")
                    O1 = opsum.tile([P, TQ], f32, tag="av")
                    Os = [O0, O1]
                    for j in range(nv):
                        for u in range(2):
                            hh = hh0 + u
                            pair, half = hh // 2, hh % 2
                            hp = slice(64 * half, 64 * half + 64)
                            S = ppool.tile([P, TQ], f32, tag="mm")
                            nc.tensor.matmul(S,
                                             lhsT=KT[hp, pair, bass.ts(j, P)],
                                             rhs=QT[hp, pair, bass.ts(s, TQ)],
                                             start=True, stop=True)
                            e = epool.tile([P, TQ], bf, tag="e")
                            if j >= 4 * s:
                                # diagonal chunk d: queries < 128d see none of
                                # these keys (e=0); queries in [128d,128d+128)
                                # need the triangular mask; rest fully visible.
                                d = j - 4 * s
                                z0 = 128 * d
                                nc.vector.tensor_tensor(
                                    S[:, z0:z0 + P], S[:, z0:z0 + P],
                                    slab, ALU.add)
                                if z0:
                                    nc.gpsimd.memset(e[:, 0:z0], 0.0)
                                nc.scalar.activation(e[:, z0:], S[:, z0:],
                                                     AF.Exp)
                            else:
                                nc.scalar.activation(e, S, AF.Exp)
                            nc.tensor.matmul(Os[u][0:65, :],
                                             lhsT=Vr[:, j, hh, :],
                                             rhs=e, start=(j == 0),
                                             stop=(j == nv - 1))
                    for u in range(2):
                        hh = hh0 + u
                        pair, half = hh // 2, hh % 2
                        hp = slice(64 * half, 64 * half + 64)
                        zr = small.tile([1, TQ], f32, tag="zr")
                        nc.vector.reciprocal(zr, Os[u][64:65, :])
                        zb = zpool.tile([64, TQ], f32, tag="zb")
                        nc.gpsimd.partition_broadcast(zb, zr)
                        nc.vector.tensor_mul(OT[hp, pair, bass.ts(s, TQ)],
                                             Os[u][0:64, :], zb)
                stg = rsst.tile([P, CC, TQ], bf, tag="stg")
                for mo in range(CC):
                    pso = ppool.tile([P, TQ], f32, tag="mm")
                    for kk in range(2):
                        nc.tensor.matmul(pso, lhsT=wo_s[:, kk, mo, :],
                                         rhs=OT[:, kk, bass.ts(s, TQ)],
                                         start=(kk == 0), stop=(kk == 1))
                    nc.vector.tensor_copy(stg[:, mo, :], pso)
                nc.sync.dma_start(rs_in[bass.ts(s, P), :, :], stg)
        ot_ctx.close()
        kvq_ctx.close()

        nc.gpsimd.collective_compute(
            "ReduceScatter", ALU.add,
            replica_groups=[[0, 1, 2, 3], [4, 5, 6, 7]],
            ins=[rs_in.opt()], outs=[rs_out.opt()])

        # ================= phase 4: residual, LN2, FFN =================
        with tc.tile_pool(name="ffp", bufs=1) as ffp, \
             tc.tile_pool(name="lnt2", bufs=2) as lnt2, \
             tc.tile_pool(name="w1p", bufs=3) as w1p, \
             tc.tile_pool(name="w2p", bufs=2) as w2p, \
             tc.tile_pool(name="fft", bufs=2) as fft:
            x_own = ffp.tile([P, CC, TQ], f32)
            nc.sync.dma_start(x_own, x_own_d)
            if USE_FP8:
                # whole W1 (4MB fp8) lands during the ReduceScatter
                w1full = ffp.tile([P, 32, 4, 2, P], fp8)
                nc.sync.dma_start(w1full, w1d)
            y1 = ffp.tile([P, CC, TQ], f32)
            rs_sb = ffp.tile([P, CC, TQ], bf)
            nc.sync.dma_start(rs_sb, rs_out)
            for mo in range(CC):
                nc.vector.scalar_tensor_tensor(y1[:, mo, :], rs_sb[:, mo, :],
                                               bo_s[:, mo:mo + 1],
                                               x_own[:, mo, :],
                                               ALU.add, ALU.add)
            A2, B2, ybs = ln_stats(y1, TQ, lnt2, "2")
            if USE_FP8:
                h2 = ffp.tile([P, 4, 2, TQ], fp8)
                z = ffp.tile([P, 16, 2, TQ], fp8)
                for cc in range(CC):
                    tt = lnt2.tile([P, TQ], bf, tag="app2")
                    nc.vector.tensor_mul(tt, ybs[cc], A2)
                    nc.vector.tensor_tensor(h2[:, cc // 2, cc % 2, :],
                                            tt, B2, ALU.subtract)
                for m in range(32):
                    psf = ppool.tile([P, TQ], f32, tag="mm")
                    for k4 in range(4):
                        nc.tensor.matmul(psf, lhsT=w1full[:, m, k4, :, :],
                                         rhs=h2[:, k4, :, :],
                                         start=(k4 == 0), stop=(k4 == 3),
                                         perf_mode=DR)
                    nc.scalar.activation(z[:, m // 2, m % 2, :], psf, AF.Relu,
                                         bias=b1_s[:, m:m + 1],
                                         scale=1.0 / W1S)
                # FFN2 in two half-contraction passes: the k16<8 pass only
                # needs z from FFN1's first 16 m-tiles, so it overlaps
                # FFN1's second half.
                ffhalf = ffp.tile([P, CC, TQ], bf)
                for mo in range(CC):
                    w2t = w2p.tile([P, 8, 2, P], fp8, tag="w2a")
                    nc.sync.dma_start(w2t, w2d[mo, :, 0:8])
                    psa = opsum.tile([P, TQ], f32, tag="av")
                    for k16 in range(8):
                        nc.tensor.matmul(psa, lhsT=w2t[:, k16, :, :],
                                         rhs=z[:, k16, :, :],
                                         start=(k16 == 0), stop=(k16 == 7),
                                         perf_mode=DR)
                    nc.vector.tensor_scalar(ffhalf[:, mo, :], psa,
                                            scalar1=1.0 / W2S,
                                            scalar2=b2_s[:, mo:mo + 1],
                                            op0=ALU.mult, op1=ALU.add)
                for mo in range(CC):
                    w2t = w2p.tile([P, 8, 2, P], fp8, tag="w2b")
                    nc.sync.dma_start(w2t, w2d[mo, :, 8:16])
                    psf = ppool.tile([P, TQ], f32, tag="mm")
                    for k16 in range(8):
                        nc.tensor.matmul(psf, lhsT=w2t[:, k16, :, :],
                                         rhs=z[:, 8 + k16, :, :],
                                         start=(k16 == 0), stop=(k16 == 7),
                                         perf_mode=DR)
                    tt = fft.tile([P, TQ], f32, tag="ep2")
                    nc.vector.tensor_scalar_mul(tt, psf, 1.0 / W2S)
                    acc = fft.tile([P, TQ], f32, tag="acc")
                    nc.vector.tensor_tensor(acc, tt, ffhalf[:, mo, :], ALU.add)
                    ot = fft.tile([P, TQ], f32, tag="ot")
                    nc.vector.tensor_tensor(ot, acc, y1[:, mo, :], ALU.add)
                    nc.sync.dma_start(outT[:, mo, :], ot)
            else:
                h2 = ffp.tile([P, CC, TQ], bf)
                z = ffp.tile([P, 32, TQ], bf)
                for cc in range(CC):
                    tt = lnt2.tile([P, TQ], bf, tag="app2")
                    nc.vector.tensor_mul(tt, ybs[cc], A2)
                    nc.vector.tensor_tensor(h2[:, cc, :], tt, B2,
                                            ALU.subtract)
                for m in range(32):
                    w1t = w1p.tile([P, CC, P], bf, tag="w1")
                    nc.sync.dma_start(w1t, w1d[m])
                    psf = ppool.tile([P, TQ], f32, tag="mm")
                    for cc in range(CC):
                        nc.tensor.matmul(psf, lhsT=w1t[:, cc, :],
                                         rhs=h2[:, cc, :],
                                         start=(cc == 0), stop=(cc == CC - 1))
                    nc.scalar.activation(z[:, m, :], psf, AF.Relu,
                                         bias=b1_s[:, m:m + 1])
                for mo in range(CC):
                    w2t = w2p.tile([P, 32, P], bf, tag="w2")
                    nc.sync.dma_start(w2t, w2d[mo])
                    psf = ppool.tile([P, TQ], f32, tag="mm")
                    for ff in range(32):
                        nc.tensor.matmul(psf, lhsT=w2t[:, ff, :],
                                         rhs=z[:, ff, :],
                                         start=(ff == 0), stop=(ff == 31))
                    tt = fft.tile([P, TQ], f32, tag="ep")
                    nc.vector.tensor_scalar_add(tt, psf, b2_s[:, mo:mo + 1])
                    ot = fft.tile([P, TQ], f32, tag="ot")
                    nc.vector.tensor_tensor(ot, tt, y1[:, mo, :], ALU.add)
                    nc.sync.dma_start(outT[:, mo, :], ot)


_NC_CACHE = {}


def build_nc(reps=1, use_fp8=None, qkv_bias=None):
    global USE_FP8, QKV_BIAS
    if use_fp8 is not None:
        USE_FP8 = use_fp8
    if qkv_bias is not None:
        QKV_BIAS = qkv_bias
    key = (reps, USE_FP8, QKV_BIAS)
    if key in _NC_CACHE:
        return _NC_CACHE[key]
    nc = bacc.Bacc("TRN2", target_bir_lowering=False, debug=False,
                   enable_asserts=False, num_devices=8)

    def dram(name, shape, dtype, kind="ExternalInput"):
        return nc.dram_tensor(name, shape, dtype, kind=kind).ap()

    if USE_FP8:
        w1_shape, w2_shape, wdt = (P, 32, 4, 2, P), (CC, P, 16, 2, P), fp8
    else:
        w1_shape, w2_shape, wdt = (32, P, CC, P), (CC, P, 32, P), bf

    aps = (
        dram("xT", (P, CC, T), f32),
        dram("x_own", (P, CC, TQ), f32),
        dram("wq", (P, CC, 2, P), bf),
        dram("wk", (P, CC, 2, P), bf),
        dram("wv", (P, CC, HPC * D), bf),
        dram("wo", (P, 2, CC, P), bf),
        dram("w1", w1_shape, wdt),
        dram("w2", w2_shape, wdt),
        dram("bq_t", (P, 2), f32),
        dram("bk_t", (P, 2), f32),
        dram("bv_bc", (P, HPC * D), f32),
        dram("bo_t", (P, CC), f32),
        dram("b1p_t", (P, 32), f32),
        dram("b2_t", (P, CC), f32),
        dram("slab", (P, SLAB_TOT), bf),
        nc.dram_tensor("rs_in", (4 * P, CC, TQ), bf).ap(),
        nc.dram_tensor("rs_out", (P, CC, TQ), bf).ap(),
        dram("outT", (P, CC, TQ), f32, kind="ExternalOutput"),
    )
    with tile.TileContext(nc) as tc:
        for _ in range(reps):
            _body(nc, tc, aps)
    nc.compile()
    _NC_CACHE[key] = nc
    return nc


def make_in_maps(inputs, use_fp8=None):
    if use_fp8 is None:
        use_fp8 = USE_FP8
    x = np.asarray(inputs["x"], np.float32)
    Wq = np.asarray(inputs["Wq"], np.float32)
    Wk = np.asarray(inputs["Wk"], np.float32)
    Wv = np.asarray(inputs["Wv"], np.float32)
    Wo = np.asarray(inputs["Wo"], np.float32)
    bo = np.asarray(inputs["bo"], np.float32)
    W1 = np.asarray(inputs["W1"], np.float32)
    b1 = np.asarray(inputs["b1"], np.float32)
    W2 = np.asarray(inputs["W2"], np.float32)
    b2 = np.asarray(inputs["b2"], np.float32)
    g1 = np.asarray(inputs["g1"], np.float32)
    be1 = np.asarray(inputs["be1"], np.float32)
    g2 = np.asarray(inputs["g2"], np.float32)
    be2 = np.asarray(inputs["be2"], np.float32)

    # fold g1 into QKV weight rows; be1 into projection biases
    Wqg = Wq * g1[None, :, None]          # (H, C, D)
    Wkg = Wk * g1[None, :, None]
    Wvg = Wv * g1[None, :, None]
    bq_h = np.einsum("c,hcd->hd", be1, Wq)   # (H, D)
    bk_h = np.einsum("c,hcd->hd", be1, Wk)
    bv_h = np.einsum("c,hcd->hd", be1, Wv)
    # fold g2 into W1 rows; be2 into b1
    W1g = W1 * g2[:, None]
    b1p = b1 + be2 @ W1                      # (FF,)

    # fp8/bf16 FFN weights
    if use_fp8:
        w1_host = np.ascontiguousarray(
            (W1S * W1g).reshape(4, 2, P, 32, P).transpose(2, 3, 0, 1, 4)
        ).astype(e4m3)                       # (P, 32, 4, 2, P)
        w2_host = np.ascontiguousarray(
            (W2S * W2).reshape(16, 2, P, CC, P).transpose(2, 0, 1, 3, 4))
        w2_host = np.ascontiguousarray(
            w2_host.transpose(3, 0, 1, 2, 4)).astype(e4m3)  # (8, P, 16, 2, P)
    else:
        w1_host = np.ascontiguousarray(
            W1g.reshape(CC, P, 32, P).transpose(2, 1, 0, 3)).astype(bf16)
        w2_host = np.ascontiguousarray(
            W2.reshape(32, P, CC, P).transpose(2, 1, 0, 3)).astype(bf16)

    # shared diagonal mask slab
    k = np.arange(P)[:, None]
    q = np.arange(P)[None, :]
    slab = np.where(k <= q, 0.0, NEG).astype(bf16)

    b2t = np.ascontiguousarray(b2.reshape(CC, P).T)
    bot = np.ascontiguousarray(bo.reshape(CC, P).T)
    b1t = np.ascontiguousarray(b1p.reshape(32, P).T)

    in_maps = []
    for c in range(8):
        b, g = c // 4, c % 4
        hs = [4 * g + i for i in range(HPC)]
        xbT = np.ascontiguousarray(x[b].T)       # (C, T)
        xT_h = np.ascontiguousarray(
            xbT.reshape(CC, P, T).transpose(1, 0, 2))
        x_own_h = np.ascontiguousarray(xT_h[:, :, TQ * g:TQ * (g + 1)])

        # wq/wk: (P, CC, pair, 128): cols = [head 2pair | head 2pair+1]
        def qk_tile(Wg):
            out = np.zeros((P, CC, 2, P), np.float32)
            for pair in range(2):
                blk = np.concatenate(
                    [Wg[hs[2 * pair]], Wg[hs[2 * pair + 1]]], axis=1)  # (C,128)
                out[:, :, pair, :] = blk.reshape(CC, P, P).transpose(1, 0, 2)
            return out.astype(bf16)

        wv_h = np.concatenate([Wvg[h] for h in hs], axis=1)   # (C, 256)
        wv_host = np.ascontiguousarray(
            wv_h.reshape(CC, P, HPC * D).transpose(1, 0, 2)).astype(bf16)

        # wo: (P, kk, mo, 128): rows 256g..256g+255 of Wo
        wo_rows = Wo[256 * g:256 * (g + 1), :]                # (256, C)
        wo_host = np.ascontiguousarray(
            wo_rows.reshape(2, P, CC, P).transpose(1, 0, 2, 3)).astype(bf16)

        bq_pair = np.concatenate(
            [np.concatenate([bq_h[hs[2 * p]], bq_h[hs[2 * p + 1]]])[:, None]
             for p in range(2)], axis=1)                      # (128, 2)
        bk_pair = np.concatenate(
            [np.concatenate([bk_h[hs[2 * p]], bk_h[hs[2 * p + 1]]])[:, None]
             for p in range(2)], axis=1)
        bv_row = np.concatenate([bv_h[h] for h in hs])        # (256,)
        bv_bc_host = np.tile(bv_row[None, :], (P, 1)).astype(np.float32)

        in_maps.append({
            "xT": xT_h,
            "x_own": x_own_h,
            "wq": qk_tile(Wqg),
            "wk": qk_tile(Wkg),
            "wv": wv_host,
            "wo": wo_host,
            "w1": w1_host,
            "w2": w2_host,
            "bq_t": np.ascontiguousarray(bq_pair * SCL),
            "bk_t": np.ascontiguousarray(bk_pair),
            "bv_bc": bv_bc_host,
            "bo_t": bot,
            "b1p_t": b1t,
            "b2_t": b2t,
            "slab": slab,
        })
    return in_maps


def assemble_output(core_outs):
    out = np.zeros((B, T, C), np.float32)
    for c in range(8):
        b, g = c // 4, c % 4
        y2 = core_outs[c]["outT"].transpose(1, 0, 2).reshape(C, TQ)
        out[b, TQ * g:TQ * (g + 1), :] = y2.T
    return out


def kernel(**inputs) -> np.ndarray:
    in_maps = make_in_maps(inputs)
    need_bias = any(
        float(np.abs(m[k]).max()) > 0.0
        for m in in_maps[:1] for k in ("bq_t", "bk_t", "bv_bc"))
    nc = build_nc(qkv_bias=need_bias)
    res = bass_utils.run_bass_kernel_spmd(nc, in_maps, core_ids=list(range(8)))
    return assemble_output(res.results)


# revision 4
# speedup vs baseline: 1.9668x; 1.0052x over previous
"""Trainium2 Bass kernel for a dense transformer block (LN -> 16-head causal
attention -> residual -> LN -> FFN -> residual) on x:(2, 2048, 1024) fp32.

Head-sharded design, 8 cores, one ReduceScatter:
  core c = (batch b=c//4, head-group g=c%4).  Each core:
    1. streams x[b] (full 2048 tokens), recomputes LN1, builds h (bf16).
    2. projects Q,K,V for ITS 4 heads over all 2048 tokens (no duplication
       across the machine; no collective).
    3. causal attention for its 4 heads: query chunks of 512 attend only to
       key chunks 0..4s+3 -- the causal triangle is identical on every core,
       so the SPMD program skips ~44%% of score/AV work with no per-core
       control flow.  Diagonal chunks get a shared additive mask slab added
       in-place in PSUM before exp.
    4. partial output projection (its 256 of 1024 contraction rows) for all
       tokens, then ONE ReduceScatter(add) over the 4-core batch group
       delivers the summed attention output for its own 512 tokens.
    5. residual + LN2 + FFN (fp8 DoubleRow matmuls) + residual for its own
       512 tokens; writes its slice of the output.

LayerNorm affine params are folded exactly: g1 into Wq/Wk/Wv rows, be1 via
projection biases; g2 into W1, be2 into the FFN1 bias.  FFN weights are
pre-scaled (x32 / x64) into fp8 range and descaled in the epilogues.
"""

import numpy as np
import ml_dtypes

import concourse.bass as bass
import concourse.tile as tile
from concourse import bacc, mybir
from concourse import bass_utils

P = 128
B, T, C = 2, 2048, 1024
H, D = 16, 64
FF = 4 * C
CC = C // P            # 8 feature chunks
TQ = 512               # own tokens per core
HPC = 4                # heads per core
NS = T // TQ           # 4 query superchunks
NT = T // P            # 16 key chunks
EPS = 1e-5
NEG = -30000.0
SCL = float(C) ** -0.5
W1S, W2S = 32.0, 64.0  # fp8 pre-scales

bf16 = ml_dtypes.bfloat16
e4m3 = ml_dtypes.float8_e4m3

f32 = mybir.dt.float32
f32r = mybir.dt.float32r
bf = mybir.dt.bfloat16
fp8 = mybir.dt.float8e4
AF = mybir.ActivationFunctionType
ALU = mybir.AluOpType
DR = mybir.MatmulPerfMode.DoubleRow

USE_FP8 = True
QKV_BIAS = False   # be1 is structurally zero in this problem's setup_inputs;
                   # kernel() switches to the biased variant if data says else
S_ORDER = (0, 1, 2, 3)

# the diagonal 128x128 causal triangle (shared by all cores/heads/chunks)
SLAB_TOT = 128


def _body(nc, tc, aps):
    (xT, x_own_d, wq, wk, wv, wo, w1d, w2d,
     bq_t, bk_t, bv_bc_d, bo_t, b1p_t, b2_t, slab_d,
     rs_in, rs_out, outT) = aps

    import contextlib
    ctx = contextlib.ExitStack()
    with ctx:
        ctx.enter_context(nc.allow_low_precision(
            reason="LN apply + softmax intermediates are bf16 by design"))
        consts = ctx.enter_context(tc.tile_pool(name="consts", bufs=1))
        ppool = ctx.enter_context(tc.tile_pool(name="ppool", bufs=4, space="PSUM"))
        opsum = ctx.enter_context(tc.tile_pool(name="opsum", bufs=3, space="PSUM"))
        spsum = ctx.enter_context(tc.tile_pool(name="spsum", bufs=1, space="PSUM"))
        small = ctx.enter_context(tc.tile_pool(name="small", bufs=2))
        lnx = ctx.enter_context(tc.tile_pool(name="lnx", bufs=2))

        def load(pool, ap_dram, shape, dtype=f32, tag=None):
            # consts go on the Act HWDGE queue so the phase-1 x-chunk
            # stream (SP queue) isn't stuck behind 1.2MB of weights
            t = pool.tile(list(shape), dtype, tag=tag or ap_dram.name)
            nc.scalar.dma_start(t, ap_dram)
            return t

        ones_f = consts.tile([P, 1], f32)
        nc.vector.memset(ones_f, 1.0)
        ones_b = consts.tile([P, 1], bf)
        nc.vector.memset(ones_b, 1.0)
        eps11 = consts.tile([1, 1], f32)
        nc.vector.memset(eps11, EPS)

        bq_s = load(consts, bq_t, (P, 2))
        bk_s = load(consts, bk_t, (P, 2))
        bv_bc = load(consts, bv_bc_d, (P, HPC * D))
        bo_s = load(consts, bo_t, (P, CC))
        b1_s = load(consts, b1p_t, (P, 32))
        b2_s = load(consts, b2_t, (P, CC))
        slab = load(consts, slab_d, (P, SLAB_TOT), bf)

        wq_s = load(consts, wq, (P, CC, 2, P), bf)
        wk_s = load(consts, wk, (P, CC, 2, P), bf)
        wv_s = load(consts, wv, (P, CC, HPC * D), bf)
        wo_s = load(consts, wo, (P, 2, CC, P), bf)

        # ---- long-lived activations
        kvq_ctx = contextlib.ExitStack()
        kvq = kvq_ctx.enter_context(tc.tile_pool(name="kvq", bufs=1))
        QT = kvq.tile([P, 2, T], bf)
        KT = kvq.tile([P, 2, T], bf)
        Vr = kvq.tile([P, NT, HPC, 65], bf)
        nc.vector.memset(Vr[:, :, :, 64:65], 1.0)

        ot_ctx = contextlib.ExitStack()
        otp = ot_ctx.enter_context(tc.tile_pool(name="otp", bufs=1))
        OT = otp.tile([P, 2, T], bf)

        def ln_stats(xs, n, lnt, tag):
            """xs: (P, CC, n) f32 in SBUF -> A_bc, B_bc (P, n) f32.
            Stats computed from a bf16 copy (Act engine) like the reference
            bf16 matmul path; walrus rejects f32r-bitcast matmuls."""
            # both stat rows share one psum bank (partitions 0 and 32)
            pspq = spsum.tile([33, n], f32, tag="pspq")
            ps = pspq[0:1, :]
            pq = pspq[32:33, :]
            xbs = []
            for cc in range(CC):
                xb = lnx.tile([P, n], bf, tag=f"xb{cc}")
                nc.scalar.activation(xb, xs[:, cc, :], AF.Copy)
                xbs.append(xb)
                sq = lnx.tile([P, n], bf, tag="sq")
                nc.vector.tensor_mul(sq, xb, xb)
                nc.tensor.matmul(ps, lhsT=ones_b, rhs=xb,
                                 start=(cc == 0), stop=(cc == CC - 1))
                nc.tensor.matmul(pq, lhsT=ones_b, rhs=sq,
                                 start=(cc == 0), stop=(cc == CC - 1))
            m = small.tile([1, n], f32, tag=f"m{tag}")
            nc.vector.tensor_scalar_mul(m, ps, 1.0 / C)
            q = small.tile([1, n], f32, tag=f"q{tag}")
            nc.vector.tensor_scalar_mul(q, pq, 1.0 / C)
            msq = small.tile([1, n], f32, tag=f"msq{tag}")
            nc.vector.tensor_mul(msq, m, m)
            nc.vector.tensor_tensor(q, q, msq, ALU.subtract)  # q := var
            sd = small.tile([1, n], f32, tag=f"sd{tag}")
            nc.scalar.activation(sd, q, AF.Sqrt, bias=eps11)
            A = small.tile([1, n], bf, tag=f"A{tag}")
            nc.vector.reciprocal(A, sd)
            Bm = small.tile([1, n], bf, tag=f"B{tag}")
            nc.vector.tensor_mul(Bm, m, A)
            A_bc = lnt.tile([P, n], bf, tag=f"Abc{tag}")
            nc.gpsimd.partition_broadcast(A_bc, A)
            B_bc = lnt.tile([P, n], bf, tag=f"Bbc{tag}")
            nc.gpsimd.partition_broadcast(B_bc, Bm)
            return A_bc, B_bc, xbs

        # ================= phase 1+2: LN1, h, Q/K/V  =================
        with tc.tile_pool(name="hpool", bufs=1) as hpool, \
             tc.tile_pool(name="xs_p", bufs=2) as xsp, \
             tc.tile_pool(name="lnt", bufs=4) as lnt:
            h = hpool.tile([P, CC, T], bf)
            for s in range(NS):
                xs = xsp.tile([P, CC, TQ], f32, tag="xs")
                # two half-DMAs: stats on cc 0-3 start before cc 4-7 land
                nc.sync.dma_start(xs[:, 0:4, :], xT[:, 0:4, bass.ts(s, TQ)])
                nc.sync.dma_start(xs[:, 4:8, :], xT[:, 4:8, bass.ts(s, TQ)])
                A_bc, B_bc, xbs = ln_stats(xs, TQ, lnt, "1")
                for cc in range(CC):
                    tt = lnt.tile([P, TQ], bf, tag="app")
                    nc.vector.tensor_mul(tt, xbs[cc], A_bc)
                    nc.vector.tensor_tensor(h[:, cc, bass.ts(s, TQ)],
                                            tt, B_bc, ALU.subtract)
                hs = h[:, :, bass.ts(s, TQ)]
                for pair in range(2):
                    psq = ppool.tile([P, TQ], f32, tag="mm")
                    for cc in range(CC):
                        nc.tensor.matmul(psq, lhsT=wq_s[:, cc, pair, :],
                                         rhs=hs[:, cc, :],
                                         start=(cc == 0), stop=(cc == CC - 1))
                    if QKV_BIAS:
                        nc.vector.tensor_scalar(QT[:, pair, bass.ts(s, TQ)],
                                                psq, scalar1=SCL,
                                                scalar2=bq_s[:, pair:pair + 1],
                                                op0=ALU.mult, op1=ALU.add)
                    else:
                        nc.scalar.activation(QT[:, pair, bass.ts(s, TQ)],
                                             psq, AF.Copy, scale=SCL)
                    psk = ppool.tile([P, TQ], f32, tag="mm")
                    for cc in range(CC):
                        nc.tensor.matmul(psk, lhsT=wk_s[:, cc, pair, :],
                                         rhs=hs[:, cc, :],
                                         start=(cc == 0), stop=(cc == CC - 1))
                    if QKV_BIAS:
                        nc.vector.tensor_scalar_add(
                            KT[:, pair, bass.ts(s, TQ)], psk,
                            bk_s[:, pair:pair + 1])
                    else:
                        nc.scalar.activation(KT[:, pair, bass.ts(s, TQ)],
                                             psk, AF.Copy)
                for jj in range(4):
                    j = 4 * s + jj
                    psvt = ppool.tile([P, TQ], f32, tag="mm")
                    psv = psvt[:, 0:HPC * D]
                    for cc in range(CC):
                        nc.tensor.matmul(psv, lhsT=hs[:, cc, bass.ts(jj, P)],
                                         rhs=wv_s[:, cc, :],
                                         start=(cc == 0), stop=(cc == CC - 1))
                    if QKV_BIAS:
                        nc.vector.tensor_tensor(
                            Vr[:, j, :, 0:64],
                            psv.rearrange("p (h d) -> p h d", d=D),
                            bv_bc.rearrange("p (h d) -> p h d", d=D), ALU.add)
                    else:
                        nc.scalar.activation(
                            Vr[:, j, :, 0:64],
                            psv.rearrange("p (h d) -> p h d", d=D), AF.Copy)

        # ================= phase 3: attention + out-proj =================
        with tc.tile_pool(name="epool", bufs=3) as epool, \
             tc.tile_pool(name="zpool", bufs=2) as zpool, \
             tc.tile_pool(name="rsst", bufs=2) as rsst:
            for s in S_ORDER:
                # start with a superchunk that only needs early K/V (overlaps
                # phase 2), end with a small one (short tail before the RS)
                nv = 4 * s + 4
                for hh0 in (0, 2):
                    # two heads interleaved: fills the scores->exp->AV latency
                    O0 = opsum.tile([P, TQ], f32, tag="av")
                    O1 = opsum.tile([P, TQ], f32, tag="av")
                    Os = [O0, O1]
                    for j in range(nv):
                        for u in range(2):
                            hh = hh0 + u
                            pair, half = hh // 2, hh % 2
                            hp = slice(64 * half, 64 * half + 64)
                            S = ppool.tile([P, TQ], f32, tag="mm")
                            nc.tensor.matmul(S,
                                             lhsT=KT[hp, pair, bass.ts(j, P)],
                                             rhs=QT[hp, pair, bass.ts(s, TQ)],
                                             start=True, stop=True)
                            e = epool.tile([P, TQ], bf, tag="e")
                            if j >= 4 * s:
                                # diagonal chunk d: queries < 128d see none of
                                # these keys (e=0); queries in [128d,128d+128)
                                # need the triangular mask; rest fully visible.
                                d = j - 4 * s
                                z0 = 128 * d
                                nc.vector.tensor_tensor(
                                    S[:, z0:z0 + P], S[:, z0:z0 + P],
                                    slab, ALU.add)
                                if z0:
                                    nc.gpsimd.memset(e[:, 0:z0], 0.0)
                                nc.scalar.activation(e[:, z0:], S[:, z0:],
                                                     AF.Exp)
                            else:
                                nc.scalar.activation(e, S, AF.Exp)
                            nc.tensor.matmul(Os[u][0:65, :],
                                             lhsT=Vr[:, j, hh, :],
                                             rhs=e, start=(j == 0),
                                             stop=(j == nv - 1))
                    for u in range(2):
                        hh = hh0 + u
                        pair, half = hh // 2, hh % 2
                        hp = slice(64 * half, 64 * half + 64)
                        zr = small.tile([1, TQ], f32, tag="zr")
                        nc.vector.reciprocal(zr, Os[u][64:65, :])
                        zb = zpool.tile([64, TQ], f32, tag="zb")
                        nc.gpsimd.partition_broadcast(zb, zr)
                        nc.vector.tensor_mul(OT[hp, pair, bass.ts(s, TQ)],
                                             Os[u][0:64, :], zb)
                stg = rsst.tile([P, CC, TQ], bf, tag="stg")
                for mo in range(CC):
                    pso = ppool.tile([P, TQ], f32, tag="mm")
                    for kk in range(2):
                        nc.tensor.matmul(pso, lhsT=wo_s[:, kk, mo, :],
                                         rhs=OT[:, kk, bass.ts(s, TQ)],
                                         start=(kk == 0), stop=(kk == 1))
                    nc.vector.tensor_copy(stg[:, mo, :], pso)
                nc.sync.dma_start(rs_in[bass.ts(s, P), :, :], stg)
        ot_ctx.close()
        kvq_ctx.close()

        nc.gpsimd.collective_compute(
            "ReduceScatter", ALU.add,
            replica_groups=[[0, 1, 2, 3], [4, 5, 6, 7]],
            ins=[rs_in.opt()], outs=[rs_out.opt()])

        # ================= phase 4: residual, LN2, FFN =================
        with tc.tile_pool(name="ffp", bufs=1) as ffp, \
             tc.tile_pool(name="lnt2", bufs=2) as lnt2, \
             tc.tile_pool(name="w1p", bufs=3) as w1p, \
             tc.tile_pool(name="w2p", bufs=2) as w2p, \
             tc.tile_pool(name="fft", bufs=2) as fft:
            x_own = ffp.tile([P, CC, TQ], f32)
            nc.sync.dma_start(x_own, x_own_d)
            if USE_FP8:
                # whole W1 (4MB fp8) lands during the ReduceScatter
                w1full = ffp.tile([P, 32, 4, 2, P], fp8)
                nc.sync.dma_start(w1full, w1d)
            y1 = ffp.tile([P, CC, TQ], f32)
            rs_sb = ffp.tile([P, CC, TQ], bf)
            nc.sync.dma_start(rs_sb, rs_out)
            for mo in range(CC):
                nc.vector.scalar_tensor_tensor(y1[:, mo, :], rs_sb[:, mo, :],
                                               bo_s[:, mo:mo + 1],
                                               x_own[:, mo, :],
                                               ALU.add, ALU.add)
            A2, B2, ybs = ln_stats(y1, TQ, lnt2, "2")
            if USE_FP8:
                h2 = ffp.tile([P, 4, 2, TQ], fp8)
                z = ffp.tile([P, 16, 2, TQ], fp8)
                for cc in range(CC):
                    tt = lnt2.tile([P, TQ], bf, tag="app2")
                    nc.vector.tensor_mul(tt, ybs[cc], A2)
                    nc.vector.tensor_tensor(h2[:, cc // 2, cc % 2, :],
                                            tt, B2, ALU.subtract)
                for m in range(32):
                    psf = ppool.tile([P, TQ], f32, tag="mm")
                    for k4 in range(4):
                        nc.tensor.matmul(psf, lhsT=w1full[:, m, k4, :, :],
                                         rhs=h2[:, k4, :, :],
                                         start=(k4 == 0), stop=(k4 == 3),
                                         perf_mode=DR)
                    nc.scalar.activation(z[:, m // 2, m % 2, :], psf, AF.Relu,
                                         bias=b1_s[:, m:m + 1],
                                         scale=1.0 / W1S)
                # FFN2 in two half-contraction passes: the k16<8 pass only
                # needs z from FFN1's first 16 m-tiles, so it overlaps
                # FFN1's second half.
                ffhalf = ffp.tile([P, CC, TQ], bf)
                for mo in range(CC):
                    w2t = w2p.tile([P, 8, 2, P], fp8, tag="w2a")
                    nc.sync.dma_start(w2t, w2d[mo, :, 0:8])
                    psa = opsum.tile([P, TQ], f32, tag="av")
                    for k16 in range(8):
                        nc.tensor.matmul(psa, lhsT=w2t[:, k16, :, :],
                                         rhs=z[:, k16, :, :],
                                         start=(k16 == 0), stop=(k16 == 7),
                                         perf_mode=DR)
                    nc.vector.tensor_scalar(ffhalf[:, mo, :], psa,
                                            scalar1=1.0 / W2S,
                                            scalar2=b2_s[:, mo:mo + 1],
                                            op0=ALU.mult, op1=ALU.add)
                for mo in range(CC):
                    w2t = w2p.tile([P, 8, 2, P], fp8, tag="w2b")
                    nc.sync.dma_start(w2t, w2d[mo, :, 8:16])
                    psf = ppool.tile([P, TQ], f32, tag="mm")
                    for k16 in range(8):
                        nc.tensor.matmul(psf, lhsT=w2t[:, k16, :, :],
                                         rhs=z[:, 8 + k16, :, :],
                                         start=(k16 == 0), stop=(k16 == 7),
                                         perf_mode=DR)
                    tt = fft.tile([P, TQ], f32, tag="ep2")
                    nc.vector.tensor_scalar_mul(tt, psf, 1.0 / W2S)
                    acc = fft.tile([P, TQ], f32, tag="acc")
                    nc.vector.tensor_tensor(acc, tt, ffhalf[:, mo, :], ALU.add)
                    ot = fft.tile([P, TQ], f32, tag="ot")
                    nc.vector.tensor_tensor(ot, acc, y1[:, mo, :], ALU.add)
                    nc.sync.dma_start(outT[:, mo, :], ot)
            else:
                h2 = ffp.tile([P, CC, TQ], bf)
                z = ffp.tile([P, 32, TQ], bf)
                for cc in range(CC):
                    tt = lnt2.tile([P, TQ], bf, tag="app2")
                    nc.vector.tensor_mul(tt, ybs[cc], A2)
                    nc.vector.tensor_tensor(h2[:, cc, :], tt, B2,
                                            ALU.subtract)
                for m in range(32):
                    w1t = w1p.tile([P, CC, P], bf, tag="w1")
                    nc.sync.dma_start(w1t, w1d[m])
                    psf = ppool.tile([P, TQ], f32, tag="mm")
                    for cc in range(CC):
                        nc.tensor.matmul(psf, lhsT=w1t[:, cc, :],
                                         rhs=h2[:, cc, :],
                                         start=(cc == 0), stop=(cc == CC - 1))
                    nc.scalar.activation(z[:, m, :], psf, AF.Relu,
                                         bias=b1_s[:, m:m + 1])
                for mo in range(CC):
                    w2t = w2p.tile([P, 32, P], bf, tag="w2")
                    nc.sync.dma_start(w2t, w2d[mo])
                    psf = ppool.tile([P, TQ], f32, tag="mm")
                    for ff in range(32):
                        nc.tensor.matmul(psf, lhsT=w2t[:, ff, :],
                                         rhs=z[:, ff, :],
                                         start=(ff == 0), stop=(ff == 31))
                    tt = fft.tile([P, TQ], f32, tag="ep")
                    nc.vector.tensor_scalar_add(tt, psf, b2_s[:, mo:mo + 1])
                    ot = fft.tile([P, TQ], f32, tag="ot")
                    nc.vector.tensor_tensor(ot, tt, y1[:, mo, :], ALU.add)
                    nc.sync.dma_start(outT[:, mo, :], ot)


_NC_CACHE = {}


def build_nc(reps=1, use_fp8=None, qkv_bias=None):
    global USE_FP8, QKV_BIAS
    if use_fp8 is not None:
        USE_FP8 = use_fp8
    if qkv_bias is not None:
        QKV_BIAS = qkv_bias
    key = (reps, USE_FP8, QKV_BIAS)
    if key in _NC_CACHE:
        return _NC_CACHE[key]
    nc = bacc.Bacc("TRN2", target_bir_lowering=False, debug=False,
                   enable_asserts=False, num_devices=8)

    def dram(name, shape, dtype, kind="ExternalInput"):
        return nc.dram_tensor(name, shape, dtype, kind=kind).ap()

    if USE_FP8:
        w1_shape, w2_shape, wdt = (P, 32, 4, 2, P), (CC, P, 16, 2, P), fp8
    else:
        w1_shape, w2_shape, wdt = (32, P, CC, P), (CC, P, 32, P), bf

    aps = (
        dram("xT", (P, CC, T), f32),
        dram("x_own", (P, CC, TQ), f32),
        dram("wq", (P, CC, 2, P), bf),
        dram("wk", (P, CC, 2, P), bf),
        dram("wv", (P, CC, HPC * D), bf),
        dram("wo", (P, 2, CC, P), bf),
        dram("w1", w1_shape, wdt),
        dram("w2", w2_shape, wdt),
        dram("bq_t", (P, 2), f32),
        dram("bk_t", (P, 2), f32),
        dram("bv_bc", (P, HPC * D), f32),
        dram("bo_t", (P, CC), f32),
        dram("b1p_t", (P, 32), f32),
        dram("b2_t", (P, CC), f32),
        dram("slab", (P, SLAB_TOT), bf),
        nc.dram_tensor("rs_in", (4 * P, CC, TQ), bf).ap(),
        nc.dram_tensor("rs_out", (P, CC, TQ), bf).ap(),
        dram("outT", (P, CC, TQ), f32, kind="ExternalOutput"),
    )
    with tile.TileContext(nc) as tc:
        for _ in range(reps):
            _body(nc, tc, aps)
    nc.compile()
    _NC_CACHE[key] = nc
    return nc


def make_in_maps(inputs, use_fp8=None):
    if use_fp8 is None:
        use_fp8 = USE_FP8
    x = np.asarray(inputs["x"], np.float32)
    Wq = np.asarray(inputs["Wq"], np.float32)
    Wk = np.asarray(inputs["Wk"], np.float32)
    Wv = np.asarray(inputs["Wv"], np.float32)
    Wo = np.asarray(inputs["Wo"], np.float32)
    bo = np.asarray(inputs["bo"], np.float32)
    W1 = np.asarray(inputs["W1"], np.float32)
    b1 = np.asarray(inputs["b1"], np.float32)
    W2 = np.asarray(inputs["W2"], np.float32)
    b2 = np.asarray(inputs["b2"], np.float32)
    g1 = np.asarray(inputs["g1"], np.float32)
    be1 = np.asarray(inputs["be1"], np.float32)
    g2 = np.asarray(inputs["g2"], np.float32)
    be2 = np.asarray(inputs["be2"], np.float32)

    # fold g1 into QKV weight rows; be1 into projection biases
    Wqg = Wq * g1[None, :, None]          # (H, C, D)
    Wkg = Wk * g1[None, :, None]
    Wvg = Wv * g1[None, :, None]
    bq_h = np.einsum("c,hcd->hd", be1, Wq)   # (H, D)
    bk_h = np.einsum("c,hcd->hd", be1, Wk)
    bv_h = np.einsum("c,hcd->hd", be1, Wv)
    # fold g2 into W1 rows; be2 into b1
    W1g = W1 * g2[:, None]
    b1p = b1 + be2 @ W1                      # (FF,)

    # fp8/bf16 FFN weights
    if use_fp8:
        w1_host = np.ascontiguousarray(
            (W1S * W1g).reshape(4, 2, P, 32, P).transpose(2, 3, 0, 1, 4)
        ).astype(e4m3)                       # (P, 32, 4, 2, P)
        w2_host = np.ascontiguousarray(
            (W2S * W2).reshape(16, 2, P, CC, P).transpose(2, 0, 1, 3, 4))
        w2_host = np.ascontiguousarray(
            w2_host.transpose(3, 0, 1, 2, 4)).astype(e4m3)  # (8, P, 16, 2, P)
    else:
        w1_host = np.ascontiguousarray(
            W1g.reshape(CC, P, 32, P).transpose(2, 1, 0, 3)).astype(bf16)
        w2_host = np.ascontiguousarray(
            W2.reshape(32, P, CC, P).transpose(2, 1, 0, 3)).astype(bf16)

    # shared diagonal mask slab
    k = np.arange(P)[:, None]
    q = np.arange(P)[None, :]
    slab = np.where(k <= q, 0.0, NEG).astype(bf16)

    b2t = np.ascontiguousarray(b2.reshape(CC, P).T)
    bot = np.ascontiguousarray(bo.reshape(CC, P).T)
    b1t = np.ascontiguousarray(b1p.reshape(32, P).T)

    in_maps = []
    for c in range(8):
        b, g = c // 4, c % 4
        hs = [4 * g + i for i in range(HPC)]
        xbT = np.ascontiguousarray(x[b].T)       # (C, T)
        xT_h = np.ascontiguousarray(
            xbT.reshape(CC, P, T).transpose(1, 0, 2))
        x_own_h = np.ascontiguousarray(xT_h[:, :, TQ * g:TQ * (g + 1)])

        # wq/wk: (P, CC, pair, 128): cols = [head 2pair | head 2pair+1]
        def qk_tile(Wg):
            out = np.zeros((P, CC, 2, P), np.float32)
            for pair in range(2):
                blk = np.concatenate(
                    [Wg[hs[2 * pair]], Wg[hs[2 * pair + 1]]], axis=1)  # (C,128)
                out[:, :, pair, :] = blk.reshape(CC, P, P).transpose(1, 0, 2)
            return out.astype(bf16)

        wv_h = np.concatenate([Wvg[h] for h in hs], axis=1)   # (C, 256)
        wv_host = np.ascontiguousarray(
            wv_h.reshape(CC, P, HPC * D).transpose(1, 0, 2)).astype(bf16)

        # wo: (P, kk, mo, 128): rows 256g..256g+255 of Wo
        wo_rows = Wo[256 * g:256 * (g + 1), :]                # (256, C)
        wo_host = np.ascontiguousarray(
            wo_rows.reshape(2, P, CC, P).transpose(1, 0, 2, 3)).astype(bf16)

        bq_pair = np.concatenate(
            [np.concatenate([bq_h[hs[2 * p]], bq_h[hs[2 * p + 1]]])[:, None]
             for p in range(2)], axis=1)                      # (128, 2)
        bk_pair = np.concatenate(
            [np.concatenate([bk_h[hs[2 * p]], bk_h[hs[2 * p + 1]]])[:, None]
             for p in range(2)], axis=1)
        bv_row = np.concatenate([bv_h[h] for h in hs])        # (256,)
        bv_bc_host = np.tile(bv_row[None, :], (P, 1)).astype(np.float32)

        in_maps.append({
            "xT": xT_h,
            "x_own": x_own_h,
            "wq": qk_tile(Wqg),
            "wk": qk_tile(Wkg),
            "wv": wv_host,
            "wo": wo_host,
            "w1": w1_host,
            "w2": w2_host,
            "bq_t": np.ascontiguousarray(bq_pair * SCL),
            "bk_t": np.ascontiguousarray(bk_pair),
            "bv_bc": bv_bc_host,
            "bo_t": bot,
            "b1p_t": b1t,
            "b2_t": b2t,
            "slab": slab,
        })
    return in_maps


def assemble_output(core_outs):
    out = np.zeros((B, T, C), np.float32)
    for c in range(8):
        b, g = c // 4, c % 4
        y2 = core_outs[c]["outT"].transpose(1, 0, 2).reshape(C, TQ)
        out[b, TQ * g:TQ * (g + 1), :] = y2.T
    return out


def kernel(**inputs) -> np.ndarray:
    in_maps = make_in_maps(inputs)
    need_bias = any(
        float(np.abs(m[k]).max()) > 0.0
        for m in in_maps[:1] for k in ("bq_t", "bk_t", "bv_bc"))
    nc = build_nc(qkv_bias=need_bias)
    res = bass_utils.run_bass_kernel_spmd(nc, in_maps, core_ids=list(range(8)))
    return assemble_output(res.results)


# revision 5
# speedup vs baseline: 1.9729x; 1.0031x over previous
"""Trainium2 Bass kernel for a dense transformer block (LN -> 16-head causal
attention -> residual -> LN -> FFN -> residual) on x:(2, 2048, 1024) fp32.

Head-sharded design, 8 cores, one ReduceScatter:
  core c = (batch b=c//4, head-group g=c%4).  Each core:
    1. streams x[b] (full 2048 tokens), recomputes LN1, builds h (bf16).
    2. projects Q,K,V for ITS 4 heads over all 2048 tokens (no duplication
       across the machine; no collective).
    3. causal attention for its 4 heads: query chunks of 512 attend only to
       key chunks 0..4s+3 -- the causal triangle is identical on every core,
       so the SPMD program skips ~44%% of score/AV work with no per-core
       control flow.  Diagonal chunks get a shared additive mask slab added
       in-place in PSUM before exp.
    4. partial output projection (its 256 of 1024 contraction rows) for all
       tokens, then ONE ReduceScatter(add) over the 4-core batch group
       delivers the summed attention output for its own 512 tokens.
    5. residual + LN2 + FFN (fp8 DoubleRow matmuls) + residual for its own
       512 tokens; writes its slice of the output.

LayerNorm affine params are folded exactly: g1 into Wq/Wk/Wv rows, be1 via
projection biases; g2 into W1, be2 into the FFN1 bias.  FFN weights are
pre-scaled (x32 / x64) into fp8 range and descaled in the epilogues.
"""

import numpy as np
import ml_dtypes

import concourse.bass as bass
import concourse.tile as tile
from concourse import bacc, mybir
from concourse import bass_utils

P = 128
B, T, C = 2, 2048, 1024
H, D = 16, 64
FF = 4 * C
CC = C // P            # 8 feature chunks
TQ = 512               # own tokens per core
HPC = 4                # heads per core
NS = T // TQ           # 4 query superchunks
NT = T // P            # 16 key chunks
EPS = 1e-5
NEG = -30000.0
SCL = float(C) ** -0.5
W1S, W2S = 32.0, 64.0  # fp8 pre-scales

bf16 = ml_dtypes.bfloat16
e4m3 = ml_dtypes.float8_e4m3

f32 = mybir.dt.float32
f32r = mybir.dt.float32r
bf = mybir.dt.bfloat16
fp8 = mybir.dt.float8e4
AF = mybir.ActivationFunctionType
ALU = mybir.AluOpType
DR = mybir.MatmulPerfMode.DoubleRow

USE_FP8 = True
QKV_BIAS = False   # be1 is structurally zero in this problem's setup_inputs;
                   # kernel() switches to the biased variant if data says else
S_ORDER = (0, 1, 2, 3)

# the diagonal 128x128 causal triangle (shared by all cores/heads/chunks)
SLAB_TOT = 128


def _body(nc, tc, aps):
    (xT, x_own_d, wq, wk, wv, wo, w1d, w2d,
     bq_t, bk_t, bv_bc_d, bo_t, b1p_t, b2_t, slab_d,
     rs_in, rs_out, outT) = aps

    import contextlib
    ctx = contextlib.ExitStack()
    with ctx:
        ctx.enter_context(nc.allow_low_precision(
            reason="LN apply + softmax intermediates are bf16 by design"))
        consts = ctx.enter_context(tc.tile_pool(name="consts", bufs=1))
        ppool = ctx.enter_context(tc.tile_pool(name="ppool", bufs=4, space="PSUM"))
        opsum = ctx.enter_context(tc.tile_pool(name="opsum", bufs=3, space="PSUM"))
        spsum = ctx.enter_context(tc.tile_pool(name="spsum", bufs=1, space="PSUM"))
        small = ctx.enter_context(tc.tile_pool(name="small", bufs=2))
        lnx = ctx.enter_context(tc.tile_pool(name="lnx", bufs=2))

        def load(pool, ap_dram, shape, dtype=f32, tag=None):
            # consts go on the Act HWDGE queue so the phase-1 x-chunk
            # stream (SP queue) isn't stuck behind 1.2MB of weights
            t = pool.tile(list(shape), dtype, tag=tag or ap_dram.name)
            nc.scalar.dma_start(t, ap_dram)
            return t

        ones_f = consts.tile([P, 1], f32)
        nc.vector.memset(ones_f, 1.0)
        ones_b = consts.tile([P, 1], bf)
        nc.vector.memset(ones_b, 1.0)
        eps11 = consts.tile([1, 1], f32)
        nc.vector.memset(eps11, EPS)

        bq_s = load(consts, bq_t, (P, 2))
        bk_s = load(consts, bk_t, (P, 2))
        bv_bc = load(consts, bv_bc_d, (P, HPC * D))
        bo_s = load(consts, bo_t, (P, CC))
        b1_s = load(consts, b1p_t, (P, 32))
        b2_s = load(consts, b2_t, (P, CC))
        slab = load(consts, slab_d, (P, SLAB_TOT), bf)

        wq_s = load(consts, wq, (P, CC, 2, P), bf)
        wk_s = load(consts, wk, (P, CC, 2, P), bf)
        wv_s = load(consts, wv, (P, CC, HPC * D), bf)
        wo_s = load(consts, wo, (P, 2, CC, P), bf)

        # ---- long-lived activations
        kvq_ctx = contextlib.ExitStack()
        kvq = kvq_ctx.enter_context(tc.tile_pool(name="kvq", bufs=1))
        QT = kvq.tile([P, 2, T], bf)
        KT = kvq.tile([P, 2, T], bf)
        Vr = kvq.tile([P, NT, HPC, 65], bf)
        nc.vector.memset(Vr[:, :, :, 64:65], 1.0)

        ot_ctx = contextlib.ExitStack()
        otp = ot_ctx.enter_context(tc.tile_pool(name="otp", bufs=1))
        OT = otp.tile([P, 2, T], bf)

        def ln_stats(xs, n, lnt, tag):
            """xs: (P, CC, n) f32 in SBUF -> A_bc, B_bc (P, n) f32.
            Stats computed from a bf16 copy (Act engine) like the reference
            bf16 matmul path; walrus rejects f32r-bitcast matmuls."""
            # both stat rows share one psum bank (partitions 0 and 32)
            pspq = spsum.tile([33, n], f32, tag="pspq")
            ps = pspq[0:1, :]
            pq = pspq[32:33, :]
            xbs = []
            for cc in range(CC):
                xb = lnx.tile([P, n], bf, tag=f"xb{cc}")
                nc.scalar.activation(xb, xs[:, cc, :], AF.Copy)
                xbs.append(xb)
                sq = lnx.tile([P, n], bf, tag="sq")
                nc.vector.tensor_mul(sq, xb, xb)
                nc.tensor.matmul(ps, lhsT=ones_b, rhs=xb,
                                 start=(cc == 0), stop=(cc == CC - 1))
                nc.tensor.matmul(pq, lhsT=ones_b, rhs=sq,
                                 start=(cc == 0), stop=(cc == CC - 1))
            m = small.tile([1, n], f32, tag=f"m{tag}")
            nc.vector.tensor_scalar_mul(m, ps, 1.0 / C)
            q = small.tile([1, n], f32, tag=f"q{tag}")
            nc.vector.tensor_scalar_mul(q, pq, 1.0 / C)
            msq = small.tile([1, n], f32, tag=f"msq{tag}")
            nc.vector.tensor_mul(msq, m, m)
            nc.vector.tensor_tensor(q, q, msq, ALU.subtract)  # q := var
            sd = small.tile([1, n], f32, tag=f"sd{tag}")
            nc.scalar.activation(sd, q, AF.Sqrt, bias=eps11)
            A = small.tile([1, n], bf, tag=f"A{tag}")
            nc.vector.reciprocal(A, sd)
            Bm = small.tile([1, n], bf, tag=f"B{tag}")
            nc.vector.tensor_mul(Bm, m, A)
            A_bc = lnt.tile([P, n], bf, tag=f"Abc{tag}")
            nc.gpsimd.partition_broadcast(A_bc, A)
            B_bc = lnt.tile([P, n], bf, tag=f"Bbc{tag}")
            nc.gpsimd.partition_broadcast(B_bc, Bm)
            return A_bc, B_bc, xbs

        # ================= phase 1+2: LN1, h, Q/K/V  =================
        with tc.tile_pool(name="hpool", bufs=1) as hpool, \
             tc.tile_pool(name="xs_p", bufs=2) as xsp, \
             tc.tile_pool(name="lnt", bufs=4) as lnt:
            h = hpool.tile([P, CC, T], bf)
            for s in range(NS):
                xs = xsp.tile([P, CC, TQ], f32, tag="xs")
                # two half-DMAs: stats on cc 0-3 start before cc 4-7 land
                nc.sync.dma_start(xs[:, 0:4, :], xT[:, 0:4, bass.ts(s, TQ)])
                nc.sync.dma_start(xs[:, 4:8, :], xT[:, 4:8, bass.ts(s, TQ)])
                A_bc, B_bc, xbs = ln_stats(xs, TQ, lnt, "1")
                for cc in range(CC):
                    tt = lnt.tile([P, TQ], bf, tag="app")
                    nc.vector.tensor_mul(tt, xbs[cc], A_bc)
                    nc.vector.tensor_tensor(h[:, cc, bass.ts(s, TQ)],
                                            tt, B_bc, ALU.subtract)
                hs = h[:, :, bass.ts(s, TQ)]
                for pair in range(2):
                    psq = ppool.tile([P, TQ], f32, tag="mm")
                    for cc in range(CC):
                        nc.tensor.matmul(psq, lhsT=wq_s[:, cc, pair, :],
                                         rhs=hs[:, cc, :],
                                         start=(cc == 0), stop=(cc == CC - 1))
                    if QKV_BIAS:
                        nc.vector.tensor_scalar(QT[:, pair, bass.ts(s, TQ)],
                                                psq, scalar1=SCL,
                                                scalar2=bq_s[:, pair:pair + 1],
                                                op0=ALU.mult, op1=ALU.add)
                    else:
                        nc.scalar.activation(QT[:, pair, bass.ts(s, TQ)],
                                             psq, AF.Copy, scale=SCL)
                    psk = ppool.tile([P, TQ], f32, tag="mm")
                    for cc in range(CC):
                        nc.tensor.matmul(psk, lhsT=wk_s[:, cc, pair, :],
                                         rhs=hs[:, cc, :],
                                         start=(cc == 0), stop=(cc == CC - 1))
                    if QKV_BIAS:
                        nc.vector.tensor_scalar_add(
                            KT[:, pair, bass.ts(s, TQ)], psk,
                            bk_s[:, pair:pair + 1])
                    else:
                        nc.scalar.activation(KT[:, pair, bass.ts(s, TQ)],
                                             psk, AF.Copy)
                for jj in range(4):
                    j = 4 * s + jj
                    psvt = ppool.tile([P, TQ], f32, tag="mm")
                    psv = psvt[:, 0:HPC * D]
                    for cc in range(CC):
                        nc.tensor.matmul(psv, lhsT=hs[:, cc, bass.ts(jj, P)],
                                         rhs=wv_s[:, cc, :],
                                         start=(cc == 0), stop=(cc == CC - 1))
                    if QKV_BIAS:
                        nc.vector.tensor_tensor(
                            Vr[:, j, :, 0:64],
                            psv.rearrange("p (h d) -> p h d", d=D),
                            bv_bc.rearrange("p (h d) -> p h d", d=D), ALU.add)
                    else:
                        nc.scalar.activation(
                            Vr[:, j, :, 0:64],
                            psv.rearrange("p (h d) -> p h d", d=D), AF.Copy)

        # ================= phase 3: attention + out-proj =================
        with tc.tile_pool(name="epool", bufs=3) as epool, \
             tc.tile_pool(name="zpool", bufs=2) as zpool, \
             tc.tile_pool(name="rsst", bufs=2) as rsst:
            for s in S_ORDER:
                # start with a superchunk that only needs early K/V (overlaps
                # phase 2), end with a small one (short tail before the RS)
                nv = 4 * s + 4
                for hh0 in (0, 2):
                    # two heads interleaved: fills the scores->exp->AV latency
                    O0 = opsum.tile([P, TQ], f32, tag="av")
                    O1 = opsum.tile([P, TQ], f32, tag="av")
                    Os = [O0, O1]
                    for j in range(nv):
                        for u in range(2):
                            hh = hh0 + u
                            pair, half = hh // 2, hh % 2
                            hp = slice(64 * half, 64 * half + 64)
                            S = ppool.tile([P, TQ], f32, tag="mm")
                            nc.tensor.matmul(S,
                                             lhsT=KT[hp, pair, bass.ts(j, P)],
                                             rhs=QT[hp, pair, bass.ts(s, TQ)],
                                             start=True, stop=True)
                            e = epool.tile([P, TQ], bf, tag="e")
                            if j >= 4 * s:
                                # diagonal chunk d: queries < 128d see none of
                                # these keys -> exp and AV cover [128d:] only;
                                # queries in [128d,128d+128) get the triangle.
                                d = j - 4 * s
                                z0 = 128 * d
                                nc.vector.tensor_tensor(
                                    S[:, z0:z0 + P], S[:, z0:z0 + P],
                                    slab, ALU.add)
                                nc.scalar.activation(e[:, z0:], S[:, z0:],
                                                     AF.Exp)
                                nc.tensor.matmul(Os[u][0:65, z0:],
                                                 lhsT=Vr[:, j, hh, :],
                                                 rhs=e[:, z0:],
                                                 start=(j == 0),
                                                 stop=(j == nv - 1))
                            else:
                                nc.scalar.activation(e, S, AF.Exp)
                                nc.tensor.matmul(Os[u][0:65, :],
                                                 lhsT=Vr[:, j, hh, :],
                                                 rhs=e, start=(j == 0),
                                                 stop=(j == nv - 1))
                    for u in range(2):
                        hh = hh0 + u
                        pair, half = hh // 2, hh % 2
                        hp = slice(64 * half, 64 * half + 64)
                        zr = small.tile([1, TQ], f32, tag="zr")
                        nc.vector.reciprocal(zr, Os[u][64:65, :])
                        zb = zpool.tile([64, TQ], f32, tag="zb")
                        nc.gpsimd.partition_broadcast(zb, zr)
                        nc.vector.tensor_mul(OT[hp, pair, bass.ts(s, TQ)],
                                             Os[u][0:64, :], zb)
                stg = rsst.tile([P, CC, TQ], bf, tag="stg")
                for mo in range(CC):
                    pso = ppool.tile([P, TQ], f32, tag="mm")
                    for kk in range(2):
                        nc.tensor.matmul(pso, lhsT=wo_s[:, kk, mo, :],
                                         rhs=OT[:, kk, bass.ts(s, TQ)],
                                         start=(kk == 0), stop=(kk == 1))
                    nc.vector.tensor_copy(stg[:, mo, :], pso)
                nc.sync.dma_start(rs_in[bass.ts(s, P), :, :], stg)
        ot_ctx.close()
        kvq_ctx.close()

        nc.gpsimd.collective_compute(
            "ReduceScatter", ALU.add,
            replica_groups=[[0, 1, 2, 3], [4, 5, 6, 7]],
            ins=[rs_in.opt()], outs=[rs_out.opt()])

        # ================= phase 4: residual, LN2, FFN =================
        with tc.tile_pool(name="ffp", bufs=1) as ffp, \
             tc.tile_pool(name="lnt2", bufs=2) as lnt2, \
             tc.tile_pool(name="w1p", bufs=3) as w1p, \
             tc.tile_pool(name="w2p", bufs=2) as w2p, \
             tc.tile_pool(name="fft", bufs=2) as fft:
            x_own = ffp.tile([P, CC, TQ], f32)
            nc.sync.dma_start(x_own, x_own_d)
            if USE_FP8:
                # whole W1 (4MB fp8) lands during the ReduceScatter
                w1full = ffp.tile([P, 32, 4, 2, P], fp8)
                nc.sync.dma_start(w1full, w1d)
            y1 = ffp.tile([P, CC, TQ], f32)
            rs_sb = ffp.tile([P, CC, TQ], bf)
            nc.sync.dma_start(rs_sb, rs_out)
            for mo in range(CC):
                nc.vector.scalar_tensor_tensor(y1[:, mo, :], rs_sb[:, mo, :],
                                               bo_s[:, mo:mo + 1],
                                               x_own[:, mo, :],
                                               ALU.add, ALU.add)
            A2, B2, ybs = ln_stats(y1, TQ, lnt2, "2")
            if USE_FP8:
                h2 = ffp.tile([P, 4, 2, TQ], fp8)
                z = ffp.tile([P, 16, 2, TQ], fp8)
                for cc in range(CC):
                    tt = lnt2.tile([P, TQ], bf, tag="app2")
                    nc.vector.tensor_mul(tt, ybs[cc], A2)
                    nc.vector.tensor_tensor(h2[:, cc // 2, cc % 2, :],
                                            tt, B2, ALU.subtract)
                for m in range(32):
                    psf = ppool.tile([P, TQ], f32, tag="mm")
                    for k4 in range(4):
                        nc.tensor.matmul(psf, lhsT=w1full[:, m, k4, :, :],
                                         rhs=h2[:, k4, :, :],
                                         start=(k4 == 0), stop=(k4 == 3),
                                         perf_mode=DR)
                    nc.scalar.activation(z[:, m // 2, m % 2, :], psf, AF.Relu,
                                         bias=b1_s[:, m:m + 1],
                                         scale=1.0 / W1S)
                # FFN2 in two half-contraction passes: the k16<8 pass only
                # needs z from FFN1's first 16 m-tiles, so it overlaps
                # FFN1's second half.
                ffhalf = ffp.tile([P, CC, TQ], bf)
                for mo in range(CC):
                    w2t = w2p.tile([P, 8, 2, P], fp8, tag="w2a")
                    nc.sync.dma_start(w2t, w2d[mo, :, 0:8])
                    psa = opsum.tile([P, TQ], f32, tag="av")
                    for k16 in range(8):
                        nc.tensor.matmul(psa, lhsT=w2t[:, k16, :, :],
                                         rhs=z[:, k16, :, :],
                                         start=(k16 == 0), stop=(k16 == 7),
                                         perf_mode=DR)
                    nc.vector.tensor_scalar(ffhalf[:, mo, :], psa,
                                            scalar1=1.0 / W2S,
                                            scalar2=b2_s[:, mo:mo + 1],
                                            op0=ALU.mult, op1=ALU.add)
                for mo in range(CC):
                    w2t = w2p.tile([P, 8, 2, P], fp8, tag="w2b")
                    nc.sync.dma_start(w2t, w2d[mo, :, 8:16])
                    psf = ppool.tile([P, TQ], f32, tag="mm")
                    for k16 in range(8):
                        nc.tensor.matmul(psf, lhsT=w2t[:, k16, :, :],
                                         rhs=z[:, 8 + k16, :, :],
                                         start=(k16 == 0), stop=(k16 == 7),
                                         perf_mode=DR)
                    tt = fft.tile([P, TQ], f32, tag="ep2")
                    nc.vector.tensor_scalar_mul(tt, psf, 1.0 / W2S)
                    acc = fft.tile([P, TQ], f32, tag="acc")
                    nc.vector.tensor_tensor(acc, tt, ffhalf[:, mo, :], ALU.add)
                    ot = fft.tile([P, TQ], f32, tag="ot")
                    nc.vector.tensor_tensor(ot, acc, y1[:, mo, :], ALU.add)
                    nc.sync.dma_start(outT[:, mo, :], ot)
            else:
                h2 = ffp.tile([P, CC, TQ], bf)
                z = ffp.tile([P, 32, TQ], bf)
                for cc in range(CC):
                    tt = lnt2.tile([P, TQ], bf, tag="app2")
                    nc.vector.tensor_mul(tt, ybs[cc], A2)
                    nc.vector.tensor_tensor(h2[:, cc, :], tt, B2,
                                            ALU.subtract)
                for m in range(32):
                    w1t = w1p.tile([P, CC, P], bf, tag="w1")
                    nc.sync.dma_start(w1t, w1d[m])
                    psf = ppool.tile([P, TQ], f32, tag="mm")
                    for cc in range(CC):
                        nc.tensor.matmul(psf, lhsT=w1t[:, cc, :],
                                         rhs=h2[:, cc, :],
                                         start=(cc == 0), stop=(cc == CC - 1))
                    nc.scalar.activation(z[:, m, :], psf, AF.Relu,
                                         bias=b1_s[:, m:m + 1])
                for mo in range(CC):
                    w2t = w2p.tile([P, 32, P], bf, tag="w2")
                    nc.sync.dma_start(w2t, w2d[mo])
                    psf = ppool.tile([P, TQ], f32, tag="mm")
                    for ff in range(32):
                        nc.tensor.matmul(psf, lhsT=w2t[:, ff, :],
                                         rhs=z[:, ff, :],
                                         start=(ff == 0), stop=(ff == 31))
                    tt = fft.tile([P, TQ], f32, tag="ep")
                    nc.vector.tensor_scalar_add(tt, psf, b2_s[:, mo:mo + 1])
                    ot = fft.tile([P, TQ], f32, tag="ot")
                    nc.vector.tensor_tensor(ot, tt, y1[:, mo, :], ALU.add)
                    nc.sync.dma_start(outT[:, mo, :], ot)


_NC_CACHE = {}


def build_nc(reps=1, use_fp8=None, qkv_bias=None):
    global USE_FP8, QKV_BIAS
    if use_fp8 is not None:
        USE_FP8 = use_fp8
    if qkv_bias is not None:
        QKV_BIAS = qkv_bias
    key = (reps, USE_FP8, QKV_BIAS)
    if key in _NC_CACHE:
        return _NC_CACHE[key]
    nc = bacc.Bacc("TRN2", target_bir_lowering=False, debug=False,
                   enable_asserts=False, num_devices=8)

    def dram(name, shape, dtype, kind="ExternalInput"):
        return nc.dram_tensor(name, shape, dtype, kind=kind).ap()

    if USE_FP8:
        w1_shape, w2_shape, wdt = (P, 32, 4, 2, P), (CC, P, 16, 2, P), fp8
    else:
        w1_shape, w2_shape, wdt = (32, P, CC, P), (CC, P, 32, P), bf

    aps = (
        dram("xT", (P, CC, T), f32),
        dram("x_own", (P, CC, TQ), f32),
        dram("wq", (P, CC, 2, P), bf),
        dram("wk", (P, CC, 2, P), bf),
        dram("wv", (P, CC, HPC * D), bf),
        dram("wo", (P, 2, CC, P), bf),
        dram("w1", w1_shape, wdt),
        dram("w2", w2_shape, wdt),
        dram("bq_t", (P, 2), f32),
        dram("bk_t", (P, 2), f32),
        dram("bv_bc", (P, HPC * D), f32),
        dram("bo_t", (P, CC), f32),
        dram("b1p_t", (P, 32), f32),
        dram("b2_t", (P, CC), f32),
        dram("slab", (P, SLAB_TOT), bf),
        nc.dram_tensor("rs_in", (4 * P, CC, TQ), bf).ap(),
        nc.dram_tensor("rs_out", (P, CC, TQ), bf).ap(),
        dram("outT", (P, CC, TQ), f32, kind="ExternalOutput"),
    )
    with tile.TileContext(nc) as tc:
        for _ in range(reps):
            _body(nc, tc, aps)
    nc.compile()
    _NC_CACHE[key] = nc
    return nc


def make_in_maps(inputs, use_fp8=None):
    if use_fp8 is None:
        use_fp8 = USE_FP8
    x = np.asarray(inputs["x"], np.float32)
    Wq = np.asarray(inputs["Wq"], np.float32)
    Wk = np.asarray(inputs["Wk"], np.float32)
    Wv = np.asarray(inputs["Wv"], np.float32)
    Wo = np.asarray(inputs["Wo"], np.float32)
    bo = np.asarray(inputs["bo"], np.float32)
    W1 = np.asarray(inputs["W1"], np.float32)
    b1 = np.asarray(inputs["b1"], np.float32)
    W2 = np.asarray(inputs["W2"], np.float32)
    b2 = np.asarray(inputs["b2"], np.float32)
    g1 = np.asarray(inputs["g1"], np.float32)
    be1 = np.asarray(inputs["be1"], np.float32)
    g2 = np.asarray(inputs["g2"], np.float32)
    be2 = np.asarray(inputs["be2"], np.float32)

    # fold g1 into QKV weight rows; be1 into projection biases
    Wqg = Wq * g1[None, :, None]          # (H, C, D)
    Wkg = Wk * g1[None, :, None]
    Wvg = Wv * g1[None, :, None]
    bq_h = np.einsum("c,hcd->hd", be1, Wq)   # (H, D)
    bk_h = np.einsum("c,hcd->hd", be1, Wk)
    bv_h = np.einsum("c,hcd->hd", be1, Wv)
    # fold g2 into W1 rows; be2 into b1
    W1g = W1 * g2[:, None]
    b1p = b1 + be2 @ W1                      # (FF,)

    # fp8/bf16 FFN weights
    if use_fp8:
        w1_host = np.ascontiguousarray(
            (W1S * W1g).reshape(4, 2, P, 32, P).transpose(2, 3, 0, 1, 4)
        ).astype(e4m3)                       # (P, 32, 4, 2, P)
        w2_host = np.ascontiguousarray(
            (W2S * W2).reshape(16, 2, P, CC, P).transpose(2, 0, 1, 3, 4))
        w2_host = np.ascontiguousarray(
            w2_host.transpose(3, 0, 1, 2, 4)).astype(e4m3)  # (8, P, 16, 2, P)
    else:
        w1_host = np.ascontiguousarray(
            W1g.reshape(CC, P, 32, P).transpose(2, 1, 0, 3)).astype(bf16)
        w2_host = np.ascontiguousarray(
            W2.reshape(32, P, CC, P).transpose(2, 1, 0, 3)).astype(bf16)

    # shared diagonal mask slab
    k = np.arange(P)[:, None]
    q = np.arange(P)[None, :]
    slab = np.where(k <= q, 0.0, NEG).astype(bf16)

    b2t = np.ascontiguousarray(b2.reshape(CC, P).T)
    bot = np.ascontiguousarray(bo.reshape(CC, P).T)
    b1t = np.ascontiguousarray(b1p.reshape(32, P).T)

    in_maps = []
    for c in range(8):
        b, g = c // 4, c % 4
        hs = [4 * g + i for i in range(HPC)]
        xbT = np.ascontiguousarray(x[b].T)       # (C, T)
        xT_h = np.ascontiguousarray(
            xbT.reshape(CC, P, T).transpose(1, 0, 2))
        x_own_h = np.ascontiguousarray(xT_h[:, :, TQ * g:TQ * (g + 1)])

        # wq/wk: (P, CC, pair, 128): cols = [head 2pair | head 2pair+1]
        def qk_tile(Wg):
            out = np.zeros((P, CC, 2, P), np.float32)
            for pair in range(2):
                blk = np.concatenate(
                    [Wg[hs[2 * pair]], Wg[hs[2 * pair + 1]]], axis=1)  # (C,128)
                out[:, :, pair, :] = blk.reshape(CC, P, P).transpose(1, 0, 2)
            return out.astype(bf16)

        wv_h = np.concatenate([Wvg[h] for h in hs], axis=1)   # (C, 256)
        wv_host = np.ascontiguousarray(
            wv_h.reshape(CC, P, HPC * D).transpose(1, 0, 2)).astype(bf16)

        # wo: (P, kk, mo, 128): rows 256g..256g+255 of Wo
        wo_rows = Wo[256 * g:256 * (g + 1), :]                # (256, C)
        wo_host = np.ascontiguousarray(
            wo_rows.reshape(2, P, CC, P).transpose(1, 0, 2, 3)).astype(bf16)

        bq_pair = np.concatenate(
            [np.concatenate([bq_h[hs[2 * p]], bq_h[hs[2 * p + 1]]])[:, None]
             for p in range(2)], axis=1)                      # (128, 2)
        bk_pair = np.concatenate(
            [np.concatenate([bk_h[hs[2 * p]], bk_h[hs[2 * p + 1]]])[:, None]
             for p in range(2)], axis=1)
        bv_row = np.concatenate([bv_h[h] for h in hs])        # (256,)
        bv_bc_host = np.tile(bv_row[None, :], (P, 1)).astype(np.float32)

        in_maps.append({
            "xT": xT_h,
            "x_own": x_own_h,
            "wq": qk_tile(Wqg),
            "wk": qk_tile(Wkg),
            "wv": wv_host,
            "wo": wo_host,
            "w1": w1_host,
            "w2": w2_host,
            "bq_t": np.ascontiguousarray(bq_pair * SCL),
            "bk_t": np.ascontiguousarray(bk_pair),
            "bv_bc": bv_bc_host,
            "bo_t": bot,
            "b1p_t": b1t,
            "b2_t": b2t,
            "slab": slab,
        })
    return in_maps


def assemble_output(core_outs):
    out = np.zeros((B, T, C), np.float32)
    for c in range(8):
        b, g = c // 4, c % 4
        y2 = core_outs[c]["outT"].transpose(1, 0, 2).reshape(C, TQ)
        out[b, TQ * g:TQ * (g + 1), :] = y2.T
    return out


def kernel(**inputs) -> np.ndarray:
    in_maps = make_in_maps(inputs)
    need_bias = any(
        float(np.abs(m[k]).max()) > 0.0
        for m in in_maps[:1] for k in ("bq_t", "bk_t", "bv_bc"))
    nc = build_nc(qkv_bias=need_bias)
    res = bass_utils.run_bass_kernel_spmd(nc, in_maps, core_ids=list(range(8)))
    return assemble_output(res.results)


# revision 7
# speedup vs baseline: 2.0466x; 1.0373x over previous
"""Trainium2 Bass kernel for a dense transformer block (LN -> 16-head causal
attention -> residual -> LN -> FFN -> residual) on x:(2, 2048, 1024) fp32.

Head-sharded design, 8 cores, one ReduceScatter:
  core c = (batch b=c//4, head-group g=c%4).  Each core:
    1. streams x[b] (full 2048 tokens), recomputes LN1, builds h (bf16).
    2. projects Q,K,V for ITS 4 heads over all 2048 tokens (no duplication
       across the machine; no collective).
    3. causal attention for its 4 heads: query chunks of 512 attend only to
       key chunks 0..4s+3 -- the causal triangle is identical on every core,
       so the SPMD program skips ~44%% of score/AV work with no per-core
       control flow.  Diagonal chunks get a shared additive mask slab added
       in-place in PSUM before exp.
    4. partial output projection (its 256 of 1024 contraction rows) for all
       tokens, then ONE ReduceScatter(add) over the 4-core batch group
       delivers the summed attention output for its own 512 tokens.
    5. residual + LN2 + FFN (fp8 DoubleRow matmuls) + residual for its own
       512 tokens; writes its slice of the output.

LayerNorm affine params are folded exactly: g1 into Wq/Wk/Wv rows, be1 via
projection biases; g2 into W1, be2 into the FFN1 bias.  FFN weights are
pre-scaled (x32 / x64) into fp8 range and descaled in the epilogues.
"""

import numpy as np
import ml_dtypes

import concourse.bass as bass
import concourse.tile as tile
from concourse import bacc, mybir
from concourse import bass_utils

P = 128
B, T, C = 2, 2048, 1024
H, D = 16, 64
FF = 4 * C
CC = C // P            # 8 feature chunks
TQ = 512               # own tokens per core
HPC = 4                # heads per core
NS = T // TQ           # 4 query superchunks
NT = T // P            # 16 key chunks
EPS = 1e-5
NEG = -30000.0
SCL = float(C) ** -0.5
W1S, W2S = 32.0, 64.0  # fp8 pre-scales
WQS = 32.0         # fp8 pre-scale for Wq/Wk/Wv

bf16 = ml_dtypes.bfloat16
e4m3 = ml_dtypes.float8_e4m3

f32 = mybir.dt.float32
f32r = mybir.dt.float32r
bf = mybir.dt.bfloat16
fp8 = mybir.dt.float8e4
AF = mybir.ActivationFunctionType
ALU = mybir.AluOpType
DR = mybir.MatmulPerfMode.DoubleRow

USE_FP8 = True
QKV_BIAS = False   # be1 is structurally zero in this problem's setup_inputs;
                   # kernel() switches to the biased variant if data says else
S_ORDER = (0, 1, 2, 3)

# the diagonal 128x128 causal triangle (shared by all cores/heads/chunks)
SLAB_TOT = 128


def _body(nc, tc, aps):
    (xT, x_own_d, wq, wk, wv, wo, w1d, w2d,
     bq_t, bk_t, bv_bc_d, bo_t, b1p_t, b2_t, slab_d,
     rs_in, rs_out, outT) = aps

    import contextlib
    ctx = contextlib.ExitStack()
    with ctx:
        ctx.enter_context(nc.allow_low_precision(
            reason="LN apply + softmax intermediates are bf16 by design"))
        consts = ctx.enter_context(tc.tile_pool(name="consts", bufs=1))
        ppool = ctx.enter_context(tc.tile_pool(name="ppool", bufs=4, space="PSUM"))
        opsum = ctx.enter_context(tc.tile_pool(name="opsum", bufs=3, space="PSUM"))
        spsum = ctx.enter_context(tc.tile_pool(name="spsum", bufs=1, space="PSUM"))
        small = ctx.enter_context(tc.tile_pool(name="small", bufs=2))
        lnx = ctx.enter_context(tc.tile_pool(name="lnx", bufs=2))

        def load(pool, ap_dram, shape, dtype=f32, tag=None):
            # consts go on the Act HWDGE queue so the phase-1 x-chunk
            # stream (SP queue) isn't stuck behind 1.2MB of weights
            t = pool.tile(list(shape), dtype, tag=tag or ap_dram.name)
            nc.scalar.dma_start(t, ap_dram)
            return t

        ones_f = consts.tile([P, 1], f32)
        nc.vector.memset(ones_f, 1.0)
        ones_b = consts.tile([P, 1], bf)
        nc.vector.memset(ones_b, 1.0)
        eps11 = consts.tile([1, 1], f32)
        nc.vector.memset(eps11, EPS)

        bq_s = load(consts, bq_t, (P, 2))
        bk_s = load(consts, bk_t, (P, 2))
        bv_bc = load(consts, bv_bc_d, (P, HPC * D))
        bo_s = load(consts, bo_t, (P, CC))
        b1_s = load(consts, b1p_t, (P, 32))
        b2_s = load(consts, b2_t, (P, CC))
        slab = load(consts, slab_d, (P, SLAB_TOT), bf)

        wq_s = load(consts, wq, (P, 4, 2, 2, P), fp8)
        wk_s = load(consts, wk, (P, 4, 2, 2, P), fp8)
        wv_s = load(consts, wv, (P, 4, 2, HPC * D), fp8)
        wo_s = load(consts, wo, (P, 2, CC, P), fp8)

        # ---- long-lived activations
        kvq_ctx = contextlib.ExitStack()
        kvq = kvq_ctx.enter_context(tc.tile_pool(name="kvq", bufs=1))
        QT = kvq.tile([P, 2, T], bf)
        KT = kvq.tile([P, 2, T], bf)
        Vr = kvq.tile([P, NT, HPC, 65], bf)
        nc.vector.memset(Vr[:, :, :, 64:65], 1.0)

        ot_ctx = contextlib.ExitStack()
        otp = ot_ctx.enter_context(tc.tile_pool(name="otp", bufs=1))
        OT = otp.tile([P, 2, T], fp8)

        def ln_stats(xs, n, lnt, tag):
            """xs: (P, CC, n) f32 in SBUF -> A_bc, B_bc (P, n) f32.
            Stats computed from a bf16 copy (Act engine) like the reference
            bf16 matmul path; walrus rejects f32r-bitcast matmuls."""
            # both stat rows share one psum bank (partitions 0 and 32)
            pspq = spsum.tile([33, n], f32, tag="pspq")
            ps = pspq[0:1, :]
            pq = pspq[32:33, :]
            xbs = []
            for cc in range(CC):
                xb = lnx.tile([P, n], bf, tag=f"xb{cc}")
                nc.scalar.activation(xb, xs[:, cc, :], AF.Copy)
                xbs.append(xb)
                sq = lnx.tile([P, n], bf, tag="sq")
                nc.vector.tensor_mul(sq, xb, xb)
                nc.tensor.matmul(ps, lhsT=ones_b, rhs=xb,
                                 start=(cc == 0), stop=(cc == CC - 1))
                nc.tensor.matmul(pq, lhsT=ones_b, rhs=sq,
                                 start=(cc == 0), stop=(cc == CC - 1))
            m = small.tile([1, n], f32, tag=f"m{tag}")
            nc.vector.tensor_scalar_mul(m, ps, 1.0 / C)
            q = small.tile([1, n], f32, tag=f"q{tag}")
            nc.vector.tensor_scalar_mul(q, pq, 1.0 / C)
            msq = small.tile([1, n], f32, tag=f"msq{tag}")
            nc.vector.tensor_mul(msq, m, m)
            nc.vector.tensor_tensor(q, q, msq, ALU.subtract)  # q := var
            sd = small.tile([1, n], f32, tag=f"sd{tag}")
            nc.scalar.activation(sd, q, AF.Sqrt, bias=eps11)
            A = small.tile([1, n], bf, tag=f"A{tag}")
            nc.vector.reciprocal(A, sd)
            Bm = small.tile([1, n], bf, tag=f"B{tag}")
            nc.vector.tensor_mul(Bm, m, A)
            A_bc = lnt.tile([P, n], bf, tag=f"Abc{tag}")
            nc.gpsimd.partition_broadcast(A_bc, A)
            B_bc = lnt.tile([P, n], bf, tag=f"Bbc{tag}")
            nc.gpsimd.partition_broadcast(B_bc, Bm)
            return A_bc, B_bc, xbs

        # ================= phase 1+2: LN1, h, Q/K/V  =================
        with tc.tile_pool(name="hpool", bufs=1) as hpool, \
             tc.tile_pool(name="xs_p", bufs=2) as xsp, \
             tc.tile_pool(name="lnt", bufs=4) as lnt:
            h = hpool.tile([P, 4, 2, T], fp8)
            for s in range(NS):
                xs = xsp.tile([P, CC, TQ], f32, tag="xs")
                # two half-DMAs: stats on cc 0-3 start before cc 4-7 land
                nc.sync.dma_start(xs[:, 0:4, :], xT[:, 0:4, bass.ts(s, TQ)])
                nc.sync.dma_start(xs[:, 4:8, :], xT[:, 4:8, bass.ts(s, TQ)])
                A_bc, B_bc, xbs = ln_stats(xs, TQ, lnt, "1")
                for cc in range(CC):
                    tt = lnt.tile([P, TQ], bf, tag="app")
                    nc.vector.tensor_mul(tt, xbs[cc], A_bc)
                    nc.vector.tensor_tensor(
                        h[:, cc // 2, cc % 2, bass.ts(s, TQ)],
                        tt, B_bc, ALU.subtract)
                hs = h[:, :, :, bass.ts(s, TQ)]
                for pair in range(2):
                    psq = ppool.tile([P, TQ], f32, tag="mm")
                    for k4 in range(4):
                        nc.tensor.matmul(psq, lhsT=wq_s[:, k4, :, pair, :],
                                         rhs=hs[:, k4, :, :],
                                         start=(k4 == 0), stop=(k4 == 3),
                                         perf_mode=DR)
                    if QKV_BIAS:
                        nc.vector.tensor_scalar(QT[:, pair, bass.ts(s, TQ)],
                                                psq, scalar1=SCL / WQS,
                                                scalar2=bq_s[:, pair:pair + 1],
                                                op0=ALU.mult, op1=ALU.add)
                    else:
                        nc.scalar.activation(QT[:, pair, bass.ts(s, TQ)],
                                             psq, AF.Copy, scale=SCL / WQS)
                    psk = ppool.tile([P, TQ], f32, tag="mm")
                    for k4 in range(4):
                        nc.tensor.matmul(psk, lhsT=wk_s[:, k4, :, pair, :],
                                         rhs=hs[:, k4, :, :],
                                         start=(k4 == 0), stop=(k4 == 3),
                                         perf_mode=DR)
                    if QKV_BIAS:
                        nc.vector.tensor_scalar(KT[:, pair, bass.ts(s, TQ)],
                                                psk, scalar1=1.0 / WQS,
                                                scalar2=bk_s[:, pair:pair + 1],
                                                op0=ALU.mult, op1=ALU.add)
                    else:
                        nc.scalar.activation(KT[:, pair, bass.ts(s, TQ)],
                                             psk, AF.Copy, scale=1.0 / WQS)
                for jj in range(4):
                    j = 4 * s + jj
                    psvt = ppool.tile([P, TQ], f32, tag="mm")
                    psv = psvt[:, 0:HPC * D]
                    for k4 in range(4):
                        nc.tensor.matmul(psv,
                                         lhsT=hs[:, k4, :, bass.ts(jj, P)],
                                         rhs=wv_s[:, k4, :, :],
                                         start=(k4 == 0), stop=(k4 == 3),
                                         perf_mode=DR)
                    if QKV_BIAS:
                        vsc = lnt.tile([P, HPC, D], f32, tag="vsc")
                        nc.vector.tensor_scalar_mul(
                            vsc, psv.rearrange("p (h d) -> p h d", d=D),
                            1.0 / WQS)
                        nc.vector.tensor_tensor(
                            Vr[:, j, :, 0:64], vsc,
                            bv_bc.rearrange("p (h d) -> p h d", d=D), ALU.add)
                    else:
                        nc.scalar.activation(
                            Vr[:, j, :, 0:64],
                            psv.rearrange("p (h d) -> p h d", d=D), AF.Copy,
                            scale=1.0 / WQS)

        # ================= phase 3: attention + out-proj =================
        with tc.tile_pool(name="epool", bufs=3) as epool, \
             tc.tile_pool(name="zpool", bufs=2) as zpool, \
             tc.tile_pool(name="rsst", bufs=2) as rsst:
            for s in S_ORDER:
                # start with a superchunk that only needs early K/V (overlaps
                # phase 2), end with a small one (short tail before the RS)
                nv = 4 * s + 4
                for hh0 in (0, 2):
                    # two heads interleaved: fills the scores->exp->AV latency
                    O0 = opsum.tile([P, TQ], f32, tag="av")
                    O1 = opsum.tile([P, TQ], f32, tag="av")
                    Os = [O0, O1]
                    for j in range(nv):
                        for u in range(2):
                            hh = hh0 + u
                            pair, half = hh // 2, hh % 2
                            hp = slice(64 * half, 64 * half + 64)
                            S = ppool.tile([P, TQ], f32, tag="mm")
                            nc.tensor.matmul(S,
                                             lhsT=KT[hp, pair, bass.ts(j, P)],
                                             rhs=QT[hp, pair, bass.ts(s, TQ)],
                                             start=True, stop=True)
                            e = epool.tile([P, TQ], bf, tag="e")
                            if j >= 4 * s:
                                # diagonal chunk d: queries < 128d see none of
                                # these keys -> exp and AV cover [128d:] only;
                                # queries in [128d,128d+128) get the triangle.
                                d = j - 4 * s
                                z0 = 128 * d
                                nc.vector.tensor_tensor(
                                    S[:, z0:z0 + P], S[:, z0:z0 + P],
                                    slab, ALU.add)
                                nc.scalar.activation(e[:, z0:], S[:, z0:],
                                                     AF.Exp)
                                nc.tensor.matmul(Os[u][0:65, z0:],
                                                 lhsT=Vr[:, j, hh, :],
                                                 rhs=e[:, z0:],
                                                 start=(j == 0),
                                                 stop=(j == nv - 1))
                            else:
                                nc.scalar.activation(e, S, AF.Exp)
                                nc.tensor.matmul(Os[u][0:65, :],
                                                 lhsT=Vr[:, j, hh, :],
                                                 rhs=e, start=(j == 0),
                                                 stop=(j == nv - 1))
                    for u in range(2):
                        hh = hh0 + u
                        pair, half = hh // 2, hh % 2
                        hp = slice(64 * half, 64 * half + 64)
                        zr = small.tile([1, TQ], f32, tag="zr")
                        nc.vector.reciprocal(zr, Os[u][64:65, :])
                        zb = zpool.tile([64, TQ], f32, tag="zb")
                        nc.gpsimd.partition_broadcast(zb, zr)
                        nc.vector.tensor_mul(OT[hp, pair, bass.ts(s, TQ)],
                                             Os[u][0:64, :], zb)
                stg = rsst.tile([P, CC, TQ], bf, tag="stg")
                for mo in range(CC):
                    pso = ppool.tile([P, TQ], f32, tag="mm")
                    nc.tensor.matmul(pso, lhsT=wo_s[:, :, mo, :],
                                     rhs=OT[:, :, bass.ts(s, TQ)],
                                     start=True, stop=True, perf_mode=DR)
                    nc.vector.tensor_scalar_mul(stg[:, mo, :], pso,
                                                1.0 / WQS)
                nc.sync.dma_start(rs_in[bass.ts(s, P), :, :], stg)
        ot_ctx.close()
        kvq_ctx.close()

        nc.gpsimd.collective_compute(
            "ReduceScatter", ALU.add,
            replica_groups=[[0, 1, 2, 3], [4, 5, 6, 7]],
            ins=[rs_in.opt()], outs=[rs_out.opt()])

        # ================= phase 4: residual, LN2, FFN =================
        with tc.tile_pool(name="ffp", bufs=1) as ffp, \
             tc.tile_pool(name="lnt2", bufs=2) as lnt2, \
             tc.tile_pool(name="w1p", bufs=3) as w1p, \
             tc.tile_pool(name="w2p", bufs=2) as w2p, \
             tc.tile_pool(name="fft", bufs=2) as fft:
            x_own = ffp.tile([P, CC, TQ], f32)
            nc.sync.dma_start(x_own, x_own_d)
            if USE_FP8:
                # whole W1 (4MB fp8) lands during the ReduceScatter
                w1full = ffp.tile([P, 32, 4, 2, P], fp8)
                nc.sync.dma_start(w1full, w1d)
            y1 = ffp.tile([P, CC, TQ], f32)
            rs_sb = ffp.tile([P, CC, TQ], bf)
            nc.sync.dma_start(rs_sb, rs_out)
            for mo in range(CC):
                nc.vector.scalar_tensor_tensor(y1[:, mo, :], rs_sb[:, mo, :],
                                               bo_s[:, mo:mo + 1],
                                               x_own[:, mo, :],
                                               ALU.add, ALU.add)
            A2, B2, ybs = ln_stats(y1, TQ, lnt2, "2")
            if USE_FP8:
                h2 = ffp.tile([P, 4, 2, TQ], fp8)
                z = ffp.tile([P, 16, 2, TQ], fp8)
                for cc in range(CC):
                    tt = lnt2.tile([P, TQ], bf, tag="app2")
                    nc.vector.tensor_mul(tt, ybs[cc], A2)
                    nc.vector.tensor_tensor(h2[:, cc // 2, cc % 2, :],
                                            tt, B2, ALU.subtract)
                for m in range(32):
                    psf = ppool.tile([P, TQ], f32, tag="mm")
                    for k4 in range(4):
                        nc.tensor.matmul(psf, lhsT=w1full[:, m, k4, :, :],
                                         rhs=h2[:, k4, :, :],
                                         start=(k4 == 0), stop=(k4 == 3),
                                         perf_mode=DR)
                    nc.scalar.activation(z[:, m // 2, m % 2, :], psf, AF.Relu,
                                         bias=b1_s[:, m:m + 1],
                                         scale=1.0 / W1S)
                # FFN2 in two half-contraction passes: the k16<8 pass only
                # needs z from FFN1's first 16 m-tiles, so it overlaps
                # FFN1's second half.
                ffhalf = ffp.tile([P, CC, TQ], bf)
                for mo in range(CC):
                    w2t = w2p.tile([P, 8, 2, P], fp8, tag="w2a")
                    nc.sync.dma_start(w2t, w2d[mo, :, 0:8])
                    psa = opsum.tile([P, TQ], f32, tag="av")
                    for k16 in range(8):
                        nc.tensor.matmul(psa, lhsT=w2t[:, k16, :, :],
                                         rhs=z[:, k16, :, :],
                                         start=(k16 == 0), stop=(k16 == 7),
                                         perf_mode=DR)
                    nc.vector.tensor_scalar(ffhalf[:, mo, :], psa,
                                            scalar1=1.0 / W2S,
                                            scalar2=b2_s[:, mo:mo + 1],
                                            op0=ALU.mult, op1=ALU.add)
                for mo in range(CC):
                    w2t = w2p.tile([P, 8, 2, P], fp8, tag="w2b")
                    nc.sync.dma_start(w2t, w2d[mo, :, 8:16])
                    psf = ppool.tile([P, TQ], f32, tag="mm")
                    for k16 in range(8):
                        nc.tensor.matmul(psf, lhsT=w2t[:, k16, :, :],
                                         rhs=z[:, 8 + k16, :, :],
                                         start=(k16 == 0), stop=(k16 == 7),
                                         perf_mode=DR)
                    tt = fft.tile([P, TQ], f32, tag="ep2")
                    nc.vector.tensor_scalar_mul(tt, psf, 1.0 / W2S)
                    acc = fft.tile([P, TQ], f32, tag="acc")
                    nc.vector.tensor_tensor(acc, tt, ffhalf[:, mo, :], ALU.add)
                    ot = fft.tile([P, TQ], f32, tag="ot")
                    nc.vector.tensor_tensor(ot, acc, y1[:, mo, :], ALU.add)
                    nc.sync.dma_start(outT[:, mo, :], ot)
            else:
                h2 = ffp.tile([P, CC, TQ], bf)
                z = ffp.tile([P, 32, TQ], bf)
                for cc in range(CC):
                    tt = lnt2.tile([P, TQ], bf, tag="app2")
                    nc.vector.tensor_mul(tt, ybs[cc], A2)
                    nc.vector.tensor_tensor(h2[:, cc, :], tt, B2,
                                            ALU.subtract)
                for m in range(32):
                    w1t = w1p.tile([P, CC, P], bf, tag="w1")
                    nc.sync.dma_start(w1t, w1d[m])
                    psf = ppool.tile([P, TQ], f32, tag="mm")
                    for cc in range(CC):
                        nc.tensor.matmul(psf, lhsT=w1t[:, cc, :],
                                         rhs=h2[:, cc, :],
                                         start=(cc == 0), stop=(cc == CC - 1))
                    nc.scalar.activation(z[:, m, :], psf, AF.Relu,
                                         bias=b1_s[:, m:m + 1])
                for mo in range(CC):
                    w2t = w2p.tile([P, 32, P], bf, tag="w2")
                    nc.sync.dma_start(w2t, w2d[mo])
                    psf = ppool.tile([P, TQ], f32, tag="mm")
                    for ff in range(32):
                        nc.tensor.matmul(psf, lhsT=w2t[:, ff, :],
                                         rhs=z[:, ff, :],
                                         start=(ff == 0), stop=(ff == 31))
                    tt = fft.tile([P, TQ], f32, tag="ep")
                    nc.vector.tensor_scalar_add(tt, psf, b2_s[:, mo:mo + 1])
                    ot = fft.tile([P, TQ], f32, tag="ot")
                    nc.vector.tensor_tensor(ot, tt, y1[:, mo, :], ALU.add)
                    nc.sync.dma_start(outT[:, mo, :], ot)


_NC_CACHE = {}


def build_nc(reps=1, use_fp8=None, qkv_bias=None):
    global USE_FP8, QKV_BIAS
    if use_fp8 is not None:
        USE_FP8 = use_fp8
    if qkv_bias is not None:
        QKV_BIAS = qkv_bias
    key = (reps, USE_FP8, QKV_BIAS)
    if key in _NC_CACHE:
        return _NC_CACHE[key]
    nc = bacc.Bacc("TRN2", target_bir_lowering=False, debug=False,
                   enable_asserts=False, num_devices=8)

    def dram(name, shape, dtype, kind="ExternalInput"):
        return nc.dram_tensor(name, shape, dtype, kind=kind).ap()

    if USE_FP8:
        w1_shape, w2_shape, wdt = (P, 32, 4, 2, P), (CC, P, 16, 2, P), fp8
    else:
        w1_shape, w2_shape, wdt = (32, P, CC, P), (CC, P, 32, P), bf

    aps = (
        dram("xT", (P, CC, T), f32),
        dram("x_own", (P, CC, TQ), f32),
        dram("wq", (P, 4, 2, 2, P), fp8),
        dram("wk", (P, 4, 2, 2, P), fp8),
        dram("wv", (P, 4, 2, HPC * D), fp8),
        dram("wo", (P, 2, CC, P), fp8),
        dram("w1", w1_shape, wdt),
        dram("w2", w2_shape, wdt),
        dram("bq_t", (P, 2), f32),
        dram("bk_t", (P, 2), f32),
        dram("bv_bc", (P, HPC * D), f32),
        dram("bo_t", (P, CC), f32),
        dram("b1p_t", (P, 32), f32),
        dram("b2_t", (P, CC), f32),
        dram("slab", (P, SLAB_TOT), bf),
        nc.dram_tensor("rs_in", (4 * P, CC, TQ), bf).ap(),
        nc.dram_tensor("rs_out", (P, CC, TQ), bf).ap(),
        dram("outT", (P, CC, TQ), f32, kind="ExternalOutput"),
    )
    with tile.TileContext(nc) as tc:
        for _ in range(reps):
            _body(nc, tc, aps)
    nc.compile()
    _NC_CACHE[key] = nc
    return nc


def make_in_maps(inputs, use_fp8=None):
    if use_fp8 is None:
        use_fp8 = USE_FP8
    x = np.asarray(inputs["x"], np.float32)
    Wq = np.asarray(inputs["Wq"], np.float32)
    Wk = np.asarray(inputs["Wk"], np.float32)
    Wv = np.asarray(inputs["Wv"], np.float32)
    Wo = np.asarray(inputs["Wo"], np.float32)
    bo = np.asarray(inputs["bo"], np.float32)
    W1 = np.asarray(inputs["W1"], np.float32)
    b1 = np.asarray(inputs["b1"], np.float32)
    W2 = np.asarray(inputs["W2"], np.float32)
    b2 = np.asarray(inputs["b2"], np.float32)
    g1 = np.asarray(inputs["g1"], np.float32)
    be1 = np.asarray(inputs["be1"], np.float32)
    g2 = np.asarray(inputs["g2"], np.float32)
    be2 = np.asarray(inputs["be2"], np.float32)

    # fold g1 into QKV weight rows; be1 into projection biases
    Wqg = Wq * g1[None, :, None]          # (H, C, D)
    Wkg = Wk * g1[None, :, None]
    Wvg = Wv * g1[None, :, None]
    bq_h = np.einsum("c,hcd->hd", be1, Wq)   # (H, D)
    bk_h = np.einsum("c,hcd->hd", be1, Wk)
    bv_h = np.einsum("c,hcd->hd", be1, Wv)
    # fold g2 into W1 rows; be2 into b1
    W1g = W1 * g2[:, None]
    b1p = b1 + be2 @ W1                      # (FF,)

    # fp8/bf16 FFN weights
    if use_fp8:
        w1_host = np.ascontiguousarray(
            (W1S * W1g).reshape(4, 2, P, 32, P).transpose(2, 3, 0, 1, 4)
        ).astype(e4m3)                       # (P, 32, 4, 2, P)
        w2_host = np.ascontiguousarray(
            (W2S * W2).reshape(16, 2, P, CC, P).transpose(2, 0, 1, 3, 4))
        w2_host = np.ascontiguousarray(
            w2_host.transpose(3, 0, 1, 2, 4)).astype(e4m3)  # (8, P, 16, 2, P)
    else:
        w1_host = np.ascontiguousarray(
            W1g.reshape(CC, P, 32, P).transpose(2, 1, 0, 3)).astype(bf16)
        w2_host = np.ascontiguousarray(
            W2.reshape(32, P, CC, P).transpose(2, 1, 0, 3)).astype(bf16)

    # shared diagonal mask slab
    k = np.arange(P)[:, None]
    q = np.arange(P)[None, :]
    slab = np.where(k <= q, 0.0, NEG).astype(bf16)

    b2t = np.ascontiguousarray(b2.reshape(CC, P).T)
    bot = np.ascontiguousarray(bo.reshape(CC, P).T)
    b1t = np.ascontiguousarray(b1p.reshape(32, P).T)

    in_maps = []
    for c in range(8):
        b, g = c // 4, c % 4
        hs = [4 * g + i for i in range(HPC)]
        xbT = np.ascontiguousarray(x[b].T)       # (C, T)
        xT_h = np.ascontiguousarray(
            xbT.reshape(CC, P, T).transpose(1, 0, 2))
        x_own_h = np.ascontiguousarray(xT_h[:, :, TQ * g:TQ * (g + 1)])

        # wq/wk: (P, CC, pair, 128): cols = [head 2pair | head 2pair+1]
        def qk_tile(Wg):
            out = np.zeros((P, 4, 2, 2, P), np.float32)
            for pair in range(2):
                blk = WQS * np.concatenate(
                    [Wg[hs[2 * pair]], Wg[hs[2 * pair + 1]]], axis=1)  # (C,128)
                out[:, :, :, pair, :] = blk.reshape(
                    4, 2, P, P).transpose(2, 0, 1, 3)
            return out.astype(e4m3)

        wv_h = WQS * np.concatenate([Wvg[h] for h in hs], axis=1)  # (C, 256)
        wv_host = np.ascontiguousarray(
            wv_h.reshape(4, 2, P, HPC * D).transpose(2, 0, 1, 3)).astype(e4m3)

        # wo: (P, kk, mo, 128): rows 256g..256g+255 of Wo
        wo_rows = Wo[256 * g:256 * (g + 1), :]                # (256, C)
        wo_host = np.ascontiguousarray(
            (WQS * wo_rows).reshape(2, P, CC, P).transpose(1, 0, 2, 3)
        ).astype(e4m3)

        bq_pair = np.concatenate(
            [np.concatenate([bq_h[hs[2 * p]], bq_h[hs[2 * p + 1]]])[:, None]
             for p in range(2)], axis=1)                      # (128, 2)
        bk_pair = np.concatenate(
            [np.concatenate([bk_h[hs[2 * p]], bk_h[hs[2 * p + 1]]])[:, None]
             for p in range(2)], axis=1)
        bv_row = np.concatenate([bv_h[h] for h in hs])        # (256,)
        bv_bc_host = np.tile(bv_row[None, :], (P, 1)).astype(np.float32)

        in_maps.append({
            "xT": xT_h,
            "x_own": x_own_h,
            "wq": qk_tile(Wqg),
            "wk": qk_tile(Wkg),
            "wv": wv_host,
            "wo": wo_host,
            "w1": w1_host,
            "w2": w2_host,
            "bq_t": np.ascontiguousarray(bq_pair * SCL),
            "bk_t": np.ascontiguousarray(bk_pair),
            "bv_bc": bv_bc_host,
            "bo_t": bot,
            "b1p_t": b1t,
            "b2_t": b2t,
            "slab": slab,
        })
    return in_maps


def assemble_output(core_outs):
    out = np.zeros((B, T, C), np.float32)
    for c in range(8):
        b, g = c // 4, c % 4
        y2 = core_outs[c]["outT"].transpose(1, 0, 2).reshape(C, TQ)
        out[b, TQ * g:TQ * (g + 1), :] = y2.T
    return out


def kernel(**inputs) -> np.ndarray:
    in_maps = make_in_maps(inputs)
    need_bias = any(
        float(np.abs(m[k]).max()) > 0.0
        for m in in_maps[:1] for k in ("bq_t", "bk_t", "bv_bc"))
    nc = build_nc(qkv_bias=need_bias)
    res = bass_utils.run_bass_kernel_spmd(nc, in_maps, core_ids=list(range(8)))
    return assemble_output(res.results)
